# revision 7
# baseline (speedup 1.0000x reference)
"""GQA attention kernel for 8 Trainium2 NeuronCores (v3, bf16 + fp8 PV).

Problem: B=2, S=2048, D=2048, 16 q-heads / 4 kv-heads (GQA), head_dim=128,
causal mask, RoPE over the full hidden dim (each head rotates with its own
frequency band), scale 1/sqrt(D), output projection.

Sharding: core c = 4*b + g handles batch b (of 2) and head-group g (of 4):
q-heads 4g..4g+3, which all share kv-head g.  The only cross-core reduction
is the output projection, summed on the host over the 4 head-groups.

v3 changes vs v2 (350us):
  - startup: one packed constants DMA; x chunk-0 split into graded pieces
    interleaved with the wq halves on the sync queue; wo load deferred to
    chunk 1; rope chunk 0 on the gpsimd queue.  First matmul ~9us, not 27.
  - off-diagonal PV + probs-sum matmuls run in fp8e4 with
    perf_mode=DoubleRow (two key-blocks contracted per matmul, 2x rate);
    the exp writes those probs tiles directly in fp8.  Diagonal blocks stay
    bf16 with trimmed widths.  PV and the sum share the same fp8 probs, so
    the normalization stays consistent.
  - causal mask add back on DVE (PE is the bottleneck); the softmax
    reciprocal moved to the scalar engine as exp(-ln(sum)) (both functions
    live in one activation table set), so no multi-us op ever sits in the
    DVE FIFO ahead of the mask adds.
"""

import sys

sys.path.insert(0, "/opt/trn_rl_repo")

from contextlib import ExitStack

import ml_dtypes
import numpy as np

import concourse.bass as bass
import concourse.tile as tile
from concourse import bacc, mybir
from concourse.bass_utils import run_bass_kernel_spmd

B, S, D = 2, 2048, 2048
NH, NG = 16, 4
KVH = NH // NG  # 4
HD = D // NH  # 128
HPC = 4  # q heads per core
ROPE_THETA = 10000.0
INV_SQRT_D = 1.0 / float(np.sqrt(np.float32(D)))
NEG = -1.0e30

F32 = mybir.dt.float32
F32R = mybir.dt.float32r
BF16 = mybir.dt.bfloat16
FP8 = mybir.dt.float8e4
BF = ml_dtypes.bfloat16
F8 = ml_dtypes.float8_e4m3

N_DT = D // 128  # 16 contraction tiles
N_SC = S // 512  # 4 seq chunks of 512
N_SB = S // 128  # 16 seq blocks of 128

EXP = mybir.ActivationFunctionType.Exp
LN = mybir.ActivationFunctionType.Ln
DR = mybir.MatmulPerfMode.DoubleRow


def build_kernel_body(ctx: ExitStack, tc: tile.TileContext, outd, ins):
    nc = tc.nc
    xr, wqr, wkr, wvr, wor, rar, rbr, constd, ons2d, ons1d = ins

    # ---------------- persistent tiles + early DMAs ----------------
    persist = ctx.enter_context(tc.tile_pool(name="persist", bufs=1))
    qt_sb = persist.tile([128, HPC, S], BF16)  # Q^T roped, per head
    kt_sb = persist.tile([128, HPC, S], BF16)  # K^T roped, per band
    v_sb = persist.tile([128, N_SB, 128], BF16)  # V s-major (diag blocks)
    v_f8 = persist.tile([128, N_SB // 2, 2, 128], FP8)  # V pairs (off-diag)
    at_sb = persist.tile([128, HPC, S], BF16)  # attn^T per head
    wo_sb = persist.tile([128, HPC, S], BF16)
    const_sb = persist.tile([128, 769], BF16)  # psw|ident|onesk|mask1
    zbias = persist.tile([128, 1], F32)
    ones2 = persist.tile([128, 2, 16], FP8)
    ones1 = persist.tile([1, 128], F32R)

    psw_sb = const_sb[:, 0:128]
    ident = const_sb[:, 128:256]
    onesk = const_sb[:, 256:257]
    mask1 = const_sb[:, 257:769]

    nc.gpsimd.memset(zbias[:], 0.0)
    nc.gpsimd.dma_start(const_sb[:], constd[:])
    nc.gpsimd.dma_start(ones2[:], ons2d[:])
    nc.gpsimd.dma_start(ones1[:], ons1d[:])

    # ---------------- phase 1: projections + RoPE ----------------
    with tc.tile_pool(name="proj_w", bufs=1) as wpool, \
         tc.tile_pool(name="xc", bufs=2) as xcpool, \
         tc.tile_pool(name="ra", bufs=2) as rapool, \
         tc.tile_pool(name="rb", bufs=2) as rbpool, \
         tc.tile_pool(name="kev", bufs=2) as kevpool, \
         tc.tile_pool(name="vev", bufs=2) as vevpool, \
         tc.tile_pool(name="qev", bufs=3) as qevpool, \
         tc.tile_pool(name="sws", bufs=3) as swspool, \
         tc.tile_pool(name="tmp", bufs=4) as tmppool, \
         tc.tile_pool(name="pacc_kv", bufs=1, space="PSUM") as pkv, \
         tc.tile_pool(name="pacc_q", bufs=1, space="PSUM") as pq, \
         tc.tile_pool(name="pswp", bufs=1, space="PSUM") as pswp_pool, \
         tc.tile_pool(name="pswq", bufs=1, space="PSUM") as pswq_pool:

        wk_sb = wpool.tile([128, N_DT, 128], BF16)
        wv_sb = wpool.tile([128, N_DT, 128], BF16)
        wq_sb = wpool.tile([128, N_DT, 512], BF16)
        nc.sync.dma_start(wk_sb[:], wkr[:])
        nc.sync.dma_start(wv_sb[:], wvr[:])

        # chunk-0 x tiles in graded pieces, wq halves interleaved
        xcs = []
        xc0 = xcpool.tile([128, N_DT, 512], BF16, name="xc_t")
        nc.sync.dma_start(xc0[:, 0:2, :], xr[:, 0, 0:2, :])
        nc.sync.dma_start(xc0[:, 2:6, :], xr[:, 0, 2:6, :])
        nc.sync.dma_start(wq_sb[:, 0:8, :], wqr[:, 0:8, :])
        nc.sync.dma_start(xc0[:, 6:12, :], xr[:, 0, 6:12, :])
        nc.sync.dma_start(wq_sb[:, 8:16, :], wqr[:, 8:16, :])
        nc.sync.dma_start(xc0[:, 12:16, :], xr[:, 0, 12:16, :])
        xcs.append(xc0)
        ra0 = rapool.tile([128, HPC, 512], BF16, name="ra_t")
        rb0 = rbpool.tile([128, HPC, 512], BF16, name="rb_t")
        nc.gpsimd.dma_start(ra0[:], rar[:, 0])
        nc.gpsimd.dma_start(rb0[:], rbr[:, 0])
        ras, rbs = [ra0], [rb0]

        for c in range(N_SC):
            cs = slice(512 * c, 512 * (c + 1))
            xc = xcs[c]
            # prefetch next chunk
            if c + 1 < N_SC:
                xcn = xcpool.tile([128, N_DT, 512], BF16, name="xc_t")
                nc.sync.dma_start(xcn[:], xr[:, c + 1])
                xcs.append(xcn)
                ran = rapool.tile([128, HPC, 512], BF16, name="ra_t")
                rbn = rbpool.tile([128, HPC, 512], BF16, name="rb_t")
                nc.scalar.dma_start(ran[:], rar[:, c + 1])
                nc.scalar.dma_start(rbn[:], rbr[:, c + 1])
                ras.append(ran)
                rbs.append(rbn)
            if c == 1:
                nc.gpsimd.dma_start(wo_sb[:], wor[:])
            ra, rb = ras[c], rbs[c]

            # K/V pass (2 psum banks)
            kv_ps = pkv.tile([128, 2, 512], F32)
            for dt in range(N_DT):
                st, sp = dt == 0, dt == N_DT - 1
                nc.tensor.matmul(kv_ps[:, 0, :], wk_sb[:, dt, :],
                                 xc[:, dt, :], start=st, stop=sp)
                nc.tensor.matmul(kv_ps[:, 1, :], wv_sb[:, dt, :],
                                 xc[:, dt, :], start=st, stop=sp)
            kraw = kevpool.tile([128, 512], BF16)
            nc.scalar.copy(kraw[:], kv_ps[:, 0, :])
            vtr = vevpool.tile([128, 512], BF16)
            nc.scalar.copy(vtr[:], kv_ps[:, 1, :])

            # Q pass (4 psum banks)
            q_ps = pq.tile([128, HPC, 512], F32)
            for dt in range(N_DT):
                st, sp = dt == 0, dt == N_DT - 1
                for i in range(HPC):
                    nc.tensor.matmul(
                        q_ps[:, i, :], wq_sb[:, dt, 128 * i:128 * (i + 1)],
                        xc[:, dt, :], start=st, stop=sp)

            # V: transpose hd-major -> s-major; dual bf16 + fp8 copies
            for j in range(4):
                kb = 4 * c + j
                vt_ps = pswq_pool.tile([128, 128], BF16, name="swq_t")
                nc.tensor.transpose(vt_ps[:],
                                    vtr[:, 128 * j:128 * (j + 1)], ident)
                nc.scalar.copy(v_sb[:, kb, :], vt_ps[:])
                nc.scalar.copy(v_f8[:, kb // 2, kb % 2, :], vt_ps[:])

            # K swap (pair-exchange along partitions) via permutation matmul
            ksw_ps = pswp_pool.tile([128, 512], F32)
            nc.tensor.matmul(ksw_ps[:], psw_sb, kraw[:],
                             start=True, stop=True)
            ksw = swspool.tile([128, 512], BF16, name="sw_t")
            nc.scalar.copy(ksw[:], ksw_ps[:])

            # RoPE per head/band
            for i in range(HPC):
                qraw = qevpool.tile([128, 512], BF16)
                nc.scalar.copy(qraw[:], q_ps[:, i, :])
                qsw_ps = pswq_pool.tile([128, 512], F32, name="swq_t")
                nc.tensor.matmul(qsw_ps[:], psw_sb, qraw[:],
                                 start=True, stop=True)
                qsw = swspool.tile([128, 512], BF16, name="sw_t")
                nc.scalar.copy(qsw[:], qsw_ps[:])

                t1 = tmppool.tile([128, 512], BF16)
                nc.vector.tensor_mul(t1[:], qraw[:], ra[:, i, :])
                t2 = tmppool.tile([128, 512], BF16)
                nc.vector.tensor_mul(t2[:], qsw[:], rb[:, i, :])
                nc.gpsimd.tensor_add(qt_sb[:, i, cs], t1[:], t2[:])

                t3 = tmppool.tile([128, 512], BF16)
                nc.vector.tensor_mul(t3[:], kraw[:], ra[:, i, :])
                t4 = tmppool.tile([128, 512], BF16)
                nc.vector.tensor_mul(t4[:], ksw[:], rb[:, i, :])
                nc.gpsimd.tensor_add(kt_sb[:, i, cs], t3[:], t4[:])

    # ---------------- phase 2: attention ----------------
    with tc.tile_pool(name="ptp", bufs=3) as ptppool, \
         tc.tile_pool(name="ptd", bufs=3) as ptdpool, \
         tc.tile_pool(name="lnv", bufs=2) as lnpool, \
         tc.tile_pool(name="rcp", bufs=2) as rcppool, \
         tc.tile_pool(name="bcs", bufs=2) as bcspool, \
         tc.tile_pool(name="ovs", bufs=3) as ovspool, \
         tc.tile_pool(name="st_ps", bufs=3, space="PSUM") as stpool, \
         tc.tile_pool(name="ov_ps", bufs=2, space="PSUM") as ovpool, \
         tc.tile_pool(name="nrm_ps", bufs=3, space="PSUM") as nrmpool:

        # Deferred normalization: stage A (evacuate ov, ln of sums, then
        # exp(-ln) -> 1/sum, all off the PE) one iteration later; stage B
        # (PE broadcast + final at_sb multiply) three iterations later.
        stage_a, stage_b = [], []

        def emit_stage_a():
            if stage_a:
                stage_a.pop(0)()

        def emit_stage_b(min_pending):
            while len(stage_b) > min_pending:
                stage_b.pop(0)()

        for qc in range(N_SC):
            for i in range(HPC):
                nkb = 4 * (qc + 1)
                npair = (4 * qc) // 2  # off-diagonal key-block pairs
                nunit = npair + 4
                ov_ps = ovpool.tile([128, 512], F32)
                sum_ps = nrmpool.tile([16, 512], F32, name="nrm_t")

                emit_stage_a()

                def emit_pair(m, pt2, ov_ps=ov_ps, sum_ps=sum_ps):
                    st = m == 0
                    nc.tensor.matmul(ov_ps[:], v_f8[:, m, :, :],
                                     pt2[:], perf_mode=DR,
                                     start=st, stop=False)
                    nc.tensor.matmul(sum_ps[:], ones2[:],
                                     pt2[:], perf_mode=DR,
                                     start=st, stop=False)

                def emit_diag(kb, ptd, qc=qc, ov_ps=ov_ps, sum_ps=sum_ps,
                              nkb=nkb):
                    o = kb - 4 * qc
                    qo = 128 * o
                    n = 512 - qo
                    st = kb == 0
                    sp = kb == nkb - 1
                    nc.tensor.matmul(ov_ps[:, qo:], v_sb[:, kb, :],
                                     ptd[:, :n], start=st, stop=sp)
                    nc.tensor.matmul(sum_ps[0:1, qo:], onesk,
                                     ptd[:, :n], start=st, stop=sp)

                prev = None  # closure awaiting PV/SUM emission
                for u in range(nunit):
                    if u < npair:
                        pt2 = ptppool.tile([128, 2, 512], FP8)
                        for j in range(2):
                            kb = 2 * u + j
                            st_ps = stpool.tile([128, 512], F32)
                            nc.tensor.matmul(
                                st_ps[:],
                                kt_sb[:, i, 128 * kb:128 * (kb + 1)],
                                qt_sb[:, i, 512 * qc:512 * (qc + 1)],
                                start=True, stop=True)
                            nc.scalar.activation(
                                pt2[:, j, :], st_ps[:], EXP,
                                bias=zbias[:], scale=INV_SQRT_D)
                        cur = (lambda u=u, pt2=pt2:
                               emit_pair(u, pt2))
                    else:
                        kb = 4 * qc + (u - npair)
                        o = u - npair
                        qo = 128 * o
                        n = 512 - qo
                        st_ps = stpool.tile([128, 512], F32)
                        nc.tensor.matmul(
                            st_ps[:, :n],
                            kt_sb[:, i, 128 * kb:128 * (kb + 1)],
                            qt_sb[:, i, 512 * qc + qo:512 * (qc + 1)],
                            start=True, stop=True)
                        nc.vector.tensor_add(st_ps[:, :n], st_ps[:, :n],
                                             mask1[:, :n])
                        ptd = ptdpool.tile([128, 512], BF16)
                        nc.scalar.activation(
                            ptd[:, :n], st_ps[:, :n], EXP,
                            bias=zbias[:], scale=INV_SQRT_D)
                        cur = (lambda kb=kb, ptd=ptd:
                               emit_diag(kb, ptd))
                    if u == 1:
                        emit_stage_b(2)
                    if prev is not None:
                        prev()
                    prev = cur
                prev()

                def a_step(i=i, qc=qc, ov_ps=ov_ps, sum_ps=sum_ps):
                    ovS = ovspool.tile([128, 512], BF16)
                    nc.vector.tensor_copy(ovS[:], ov_ps[:])
                    lnv = lnpool.tile([1, 512], F32)
                    nc.scalar.activation(lnv[:], sum_ps[0:1, :], LN,
                                         bias=zbias[0:1, :])
                    rcp = rcppool.tile([1, 512], F32R)
                    with nc.allow_low_precision(
                            reason="f32r recip feeds matmul rhs"):
                        nc.scalar.activation(rcp[:], lnv[:], EXP,
                                             bias=zbias[0:1, :], scale=-1.0)

                    def b_step(i=i, qc=qc, ovS=ovS, rcp=rcp):
                        bc_ps = nrmpool.tile([128, 512], F32, name="nrm_t")
                        nc.tensor.matmul(bc_ps[:], ones1[:], rcp[:],
                                         start=True, stop=True)
                        bcS = bcspool.tile([128, 512], BF16)
                        nc.vector.tensor_copy(bcS[:], bc_ps[:])
                        nc.vector.tensor_mul(
                            at_sb[:, i, 512 * qc:512 * (qc + 1)],
                            ovS[:], bcS[:])

                    stage_b.append(b_step)

                stage_a.append(a_step)

        emit_stage_a()
        emit_stage_b(0)

    # ---------------- phase 3: output projection ----------------
    with tc.tile_pool(name="osb", bufs=3) as opool, \
         tc.tile_pool(name="op_ps", bufs=3, space="PSUM") as oppool:
        for sc in range(N_SC):
            ss = slice(512 * sc, 512 * (sc + 1))
            for jb in range(N_SB):
                op_ps = oppool.tile([128, 512], F32)
                for h in range(HPC):
                    nc.tensor.matmul(
                        op_ps[:], wo_sb[:, h, 128 * jb:128 * (jb + 1)],
                        at_sb[:, h, ss],
                        start=(h == 0), stop=(h == HPC - 1))
                osb = opool.tile([128, 512], BF16)
                if jb % 2 == 0:
                    nc.scalar.copy(osb[:], op_ps[:])
                else:
                    nc.vector.tensor_copy(osb[:], op_ps[:])
                nc.sync.dma_start(outd[:, jb, sc, :], osb[:])


_NC_CACHE = None


def get_nc():
    global _NC_CACHE
    if _NC_CACHE is not None:
        return _NC_CACHE
    nc = bacc.Bacc("TRN2", target_bir_lowering=False, debug=False,
                   num_devices=8)
    xr = nc.dram_tensor("xr", [128, N_SC, N_DT, 512], BF16,
                        kind="ExternalInput").ap()
    wqr = nc.dram_tensor("wqr", [128, N_DT, 512], BF16,
                         kind="ExternalInput").ap()
    wkr = nc.dram_tensor("wkr", [128, N_DT, 128], BF16,
                         kind="ExternalInput").ap()
    wvr = nc.dram_tensor("wvr", [128, N_DT, 128], BF16,
                         kind="ExternalInput").ap()
    wor = nc.dram_tensor("wor", [128, HPC, S], BF16,
                         kind="ExternalInput").ap()
    rar = nc.dram_tensor("rar", [128, N_SC, HPC, 512], BF16,
                         kind="ExternalInput").ap()
    rbr = nc.dram_tensor("rbr", [128, N_SC, HPC, 512], BF16,
                         kind="ExternalInput").ap()
    constd = nc.dram_tensor("constd", [128, 769], BF16,
                            kind="ExternalInput").ap()
    ons2d = nc.dram_tensor("ons2d", [128, 2, 16], FP8,
                           kind="ExternalInput").ap()
    ons1d = nc.dram_tensor("ons1d", [1, 128], F32R,
                           kind="ExternalInput").ap()
    outd = nc.dram_tensor("outd", [128, N_SB, N_SC, 512], BF16,
                          kind="ExternalOutput").ap()

    with tile.TileContext(nc) as tc, ExitStack() as ctx:
        build_kernel_body(ctx, tc, outd,
                          (xr, wqr, wkr, wvr, wor, rar, rbr, constd,
                           ons2d, ons1d))
    nc.compile()
    _NC_CACHE = nc
    return nc


def host_inputs(x, Wq, Wk, Wv, Wo):
    """Per-core input dicts (core c = 4*b + g), pre-arranged + cast."""
    x = np.asarray(x, np.float32)
    Wq = np.asarray(Wq, np.float32)
    Wk = np.asarray(Wk, np.float32)
    Wv = np.asarray(Wv, np.float32)
    Wo = np.asarray(Wo, np.float32)

    # rope tables (same freqs layout as the reference)
    freqs = 1.0 / (ROPE_THETA ** (np.arange(0, D, 2, dtype=np.float32) / D))
    ang = np.arange(S, dtype=np.float32)[:, None] * freqs[None, :]  # [S, D/2]
    cos = np.cos(ang).astype(np.float32)
    sin = np.sin(ang).astype(np.float32)
    sgn = np.where(np.arange(128) % 2 == 0, -1.0, 1.0).astype(np.float32)

    # packed constants: psw | ident | onesk | mask1
    pswap = np.zeros((128, 128), np.float32)
    idx = np.arange(128)
    pswap[idx, idx ^ 1] = 1.0
    p = np.arange(128)[:, None]
    f = np.arange(512)[None, :]
    mask1 = np.where(p > f, np.float32(NEG), np.float32(0.0))
    constd = np.concatenate(
        [pswap, np.eye(128, dtype=np.float32),
         np.ones((128, 1), np.float32), mask1], axis=1).astype(BF)

    xrs = [np.ascontiguousarray(
        x[b].reshape(N_SC, 512, N_DT, 128).transpose(3, 0, 2, 1)).astype(BF)
        for b in range(B)]

    in_maps = []
    for c in range(8):
        b, g = divmod(c, 4)
        wqr = Wq[512 * g:512 * (g + 1)].reshape(512, N_DT, 128).transpose(
            2, 1, 0).astype(BF)
        wkr = Wk[128 * g:128 * (g + 1)].reshape(128, N_DT, 128).transpose(
            2, 1, 0).astype(BF)
        wvr = Wv[128 * g:128 * (g + 1)].reshape(128, N_DT, 128).transpose(
            2, 1, 0).astype(BF)
        wor = Wo[:, 512 * g:512 * (g + 1)].reshape(S, HPC, 128).transpose(
            2, 1, 0).astype(BF)
        rar = np.empty((128, N_SC, HPC, 512), np.float32)
        rbr = np.empty((128, N_SC, HPC, 512), np.float32)
        for i in range(HPC):
            fidx = 256 * g + 64 * i + (np.arange(128) // 2)  # [128]
            band_a = cos[:, fidx].T  # [128, S]
            band_b = sin[:, fidx].T * sgn[:, None]
            rar[:, :, i, :] = band_a.reshape(128, N_SC, 512)
            rbr[:, :, i, :] = band_b.reshape(128, N_SC, 512)
        in_maps.append({
            "xr": xrs[b],
            "wqr": np.ascontiguousarray(wqr),
            "wkr": np.ascontiguousarray(wkr),
            "wvr": np.ascontiguousarray(wvr),
            "wor": np.ascontiguousarray(wor),
            "rar": rar.astype(BF),
            "rbr": rbr.astype(BF),
            "constd": constd,
            "ons2d": np.ones((128, 2, 16), np.float32).astype(F8),
            "ons1d": np.ones((1, 128), np.float32),
        })
    return in_maps


def kernel(x, Wq, Wk, Wv, Wo, mask, _trace=False):
    in_maps = host_inputs(x, Wq, Wk, Wv, Wo)
    nc = get_nc()
    res = run_bass_kernel_spmd(nc, in_maps, list(range(8)), trace=_trace)
    # outd [128, jb, sc, 512] -> partial [D, S]
    outs = [np.asarray(res.results[c]["outd"], dtype=np.float32)
            .transpose(1, 0, 2, 3).reshape(D, S) for c in range(8)]
    out = np.stack([
        (outs[4 * b + 0] + outs[4 * b + 1] + outs[4 * b + 2]
         + outs[4 * b + 3]).T
        for b in range(B)
    ]).astype(np.float32)
    if _trace:
        kernel.last_result = res
    return out


# revision 8
# speedup vs baseline: 1.2290x; 1.2290x over previous
"""GQA attention kernel for 8 Trainium2 NeuronCores (v3, bf16 + fp8 PV).

Problem: B=2, S=2048, D=2048, 16 q-heads / 4 kv-heads (GQA), head_dim=128,
causal mask, RoPE over the full hidden dim (each head rotates with its own
frequency band), scale 1/sqrt(D), output projection.

Sharding: core c = 4*b + g handles batch b (of 2) and head-group g (of 4):
q-heads 4g..4g+3, which all share kv-head g.  The only cross-core reduction
is the output projection, summed on the host over the 4 head-groups.

v3 changes vs v2 (350us):
  - startup: one packed constants DMA; x chunk-0 split into graded pieces
    interleaved with the wq halves on the sync queue; wo load deferred to
    chunk 1; rope chunk 0 on the gpsimd queue.  First matmul ~9us, not 27.
  - off-diagonal PV + probs-sum matmuls run in fp8e4 with
    perf_mode=DoubleRow (two key-blocks contracted per matmul, 2x rate);
    the exp writes those probs tiles directly in fp8.  Diagonal blocks stay
    bf16 with trimmed widths.  PV and the sum share the same fp8 probs, so
    the normalization stays consistent.
  - causal mask add back on DVE (PE is the bottleneck); the softmax
    reciprocal moved to the scalar engine as exp(-ln(sum)) (both functions
    live in one activation table set), so no multi-us op ever sits in the
    DVE FIFO ahead of the mask adds.
"""

import sys

sys.path.insert(0, "/opt/trn_rl_repo")

from contextlib import ExitStack

import ml_dtypes
import numpy as np

import concourse.bass as bass
import concourse.tile as tile
from concourse import bacc, mybir
from concourse.bass_utils import run_bass_kernel_spmd

B, S, D = 2, 2048, 2048
NH, NG = 16, 4
KVH = NH // NG  # 4
HD = D // NH  # 128
HPC = 4  # q heads per core
ROPE_THETA = 10000.0
INV_SQRT_D = 1.0 / float(np.sqrt(np.float32(D)))
NEG = -1.0e30

F32 = mybir.dt.float32
F32R = mybir.dt.float32r
BF16 = mybir.dt.bfloat16
FP8 = mybir.dt.float8e4
BF = ml_dtypes.bfloat16
F8 = ml_dtypes.float8_e4m3

N_DT = D // 128  # 16 contraction tiles
N_SC = S // 512  # 4 seq chunks of 512
N_SB = S // 128  # 16 seq blocks of 128

EXP = mybir.ActivationFunctionType.Exp
LN = mybir.ActivationFunctionType.Ln
DR = mybir.MatmulPerfMode.DoubleRow


def build_kernel_body(ctx: ExitStack, tc: tile.TileContext, outd, ins):
    nc = tc.nc
    xr, wqr, wkr, wvr, wor, rar, rbr, constd, ons2d, ons1d = ins

    # ---------------- persistent tiles + early DMAs ----------------
    persist = ctx.enter_context(tc.tile_pool(name="persist", bufs=1))
    qt_sb = persist.tile([128, HPC, S], BF16)  # Q^T roped, per head
    kt_sb = persist.tile([128, HPC, S], BF16)  # K^T roped, per band
    v_sb = persist.tile([128, N_SB, 128], BF16)  # V s-major (diag blocks)
    v_f8 = persist.tile([128, N_SB // 2, 2, 128], FP8)  # V pairs (off-diag)
    at_sb = persist.tile([128, HPC, S], BF16)  # attn^T per head
    wo_sb = persist.tile([128, HPC, S], BF16)
    const_sb = persist.tile([128, 769], BF16)  # psw|ident|onesk|mask1
    zbias = persist.tile([128, 1], F32)
    ones2 = persist.tile([128, 2, 16], FP8)
    ones1 = persist.tile([1, 128], F32R)

    psw_sb = const_sb[:, 0:128]
    ident = const_sb[:, 128:256]
    onesk = const_sb[:, 256:257]
    mask1 = const_sb[:, 257:769]

    nc.gpsimd.memset(zbias[:], 0.0)
    nc.gpsimd.dma_start(const_sb[:], constd[:])
    nc.gpsimd.dma_start(ones2[:], ons2d[:])
    nc.gpsimd.dma_start(ones1[:], ons1d[:])

    # ---------------- phase 1: projections + RoPE ----------------
    with tc.tile_pool(name="proj_w", bufs=1) as wpool, \
         tc.tile_pool(name="xc", bufs=2) as xcpool, \
         tc.tile_pool(name="ra", bufs=2) as rapool, \
         tc.tile_pool(name="rb", bufs=2) as rbpool, \
         tc.tile_pool(name="kev", bufs=2) as kevpool, \
         tc.tile_pool(name="vev", bufs=2) as vevpool, \
         tc.tile_pool(name="qev", bufs=3) as qevpool, \
         tc.tile_pool(name="sws", bufs=3) as swspool, \
         tc.tile_pool(name="tmp", bufs=4) as tmppool, \
         tc.tile_pool(name="pacc_kv", bufs=1, space="PSUM") as pkv, \
         tc.tile_pool(name="pacc_q", bufs=1, space="PSUM") as pq, \
         tc.tile_pool(name="pswp", bufs=1, space="PSUM") as pswp_pool, \
         tc.tile_pool(name="pswq", bufs=1, space="PSUM") as pswq_pool:

        wk_sb = wpool.tile([128, N_DT, 128], BF16)
        wv_sb = wpool.tile([128, N_DT, 128], BF16)
        wq_sb = wpool.tile([128, N_DT, 512], BF16)
        nc.sync.dma_start(wk_sb[:], wkr[:])
        nc.sync.dma_start(wv_sb[:], wvr[:])

        # chunk-0 x tiles in graded pieces, wq halves interleaved
        xcs = []
        xc0 = xcpool.tile([128, N_DT, 512], BF16, name="xc_t")
        nc.sync.dma_start(xc0[:, 0:2, :], xr[:, 0, 0:2, :])
        nc.sync.dma_start(xc0[:, 2:6, :], xr[:, 0, 2:6, :])
        nc.sync.dma_start(wq_sb[:, 0:8, :], wqr[:, 0:8, :])
        nc.sync.dma_start(xc0[:, 6:12, :], xr[:, 0, 6:12, :])
        nc.sync.dma_start(wq_sb[:, 8:16, :], wqr[:, 8:16, :])
        nc.sync.dma_start(xc0[:, 12:16, :], xr[:, 0, 12:16, :])
        xcs.append(xc0)
        ra0 = rapool.tile([128, HPC, 512], BF16, name="ra_t")
        rb0 = rbpool.tile([128, HPC, 512], BF16, name="rb_t")
        nc.gpsimd.dma_start(ra0[:], rar[:, 0])
        nc.gpsimd.dma_start(rb0[:], rbr[:, 0])
        ras, rbs = [ra0], [rb0]

        for c in range(N_SC):
            cs = slice(512 * c, 512 * (c + 1))
            xc = xcs[c]
            # prefetch next chunk
            if c + 1 < N_SC:
                xcn = xcpool.tile([128, N_DT, 512], BF16, name="xc_t")
                nc.sync.dma_start(xcn[:], xr[:, c + 1])
                xcs.append(xcn)
                ran = rapool.tile([128, HPC, 512], BF16, name="ra_t")
                rbn = rbpool.tile([128, HPC, 512], BF16, name="rb_t")
                nc.scalar.dma_start(ran[:], rar[:, c + 1])
                nc.scalar.dma_start(rbn[:], rbr[:, c + 1])
                ras.append(ran)
                rbs.append(rbn)
            if c == 1:
                nc.gpsimd.dma_start(wo_sb[:], wor[:])
            ra, rb = ras[c], rbs[c]

            # K/V pass (2 psum banks)
            kv_ps = pkv.tile([128, 2, 512], F32)
            for dt in range(N_DT):
                st, sp = dt == 0, dt == N_DT - 1
                nc.tensor.matmul(kv_ps[:, 0, :], wk_sb[:, dt, :],
                                 xc[:, dt, :], start=st, stop=sp)
                nc.tensor.matmul(kv_ps[:, 1, :], wv_sb[:, dt, :],
                                 xc[:, dt, :], start=st, stop=sp)
            kraw = kevpool.tile([128, 512], BF16)
            nc.scalar.copy(kraw[:], kv_ps[:, 0, :])
            vtr = vevpool.tile([128, 512], BF16)
            nc.scalar.copy(vtr[:], kv_ps[:, 1, :])

            # Q pass (4 psum banks)
            q_ps = pq.tile([128, HPC, 512], F32)
            for dt in range(N_DT):
                st, sp = dt == 0, dt == N_DT - 1
                for i in range(HPC):
                    nc.tensor.matmul(
                        q_ps[:, i, :], wq_sb[:, dt, 128 * i:128 * (i + 1)],
                        xc[:, dt, :], start=st, stop=sp)

            # V: transpose hd-major -> s-major; dual bf16 + fp8 copies
            for j in range(4):
                kb = 4 * c + j
                vt_ps = pswq_pool.tile([128, 128], BF16, name="swq_t")
                nc.tensor.transpose(vt_ps[:],
                                    vtr[:, 128 * j:128 * (j + 1)], ident)
                nc.scalar.copy(v_sb[:, kb, :], vt_ps[:])
                nc.scalar.copy(v_f8[:, kb // 2, kb % 2, :], vt_ps[:])

            # K swap (pair-exchange along partitions) via permutation matmul
            ksw_ps = pswp_pool.tile([128, 512], F32)
            nc.tensor.matmul(ksw_ps[:], psw_sb, kraw[:],
                             start=True, stop=True)
            ksw = swspool.tile([128, 512], BF16, name="sw_t")
            nc.scalar.copy(ksw[:], ksw_ps[:])

            # RoPE per head/band
            for i in range(HPC):
                qraw = qevpool.tile([128, 512], BF16)
                nc.scalar.copy(qraw[:], q_ps[:, i, :])
                qsw_ps = pswq_pool.tile([128, 512], F32, name="swq_t")
                nc.tensor.matmul(qsw_ps[:], psw_sb, qraw[:],
                                 start=True, stop=True)
                qsw = swspool.tile([128, 512], BF16, name="sw_t")
                nc.scalar.copy(qsw[:], qsw_ps[:])

                t1 = tmppool.tile([128, 512], BF16)
                nc.vector.tensor_mul(t1[:], qraw[:], ra[:, i, :])
                t2 = tmppool.tile([128, 512], BF16)
                nc.vector.tensor_mul(t2[:], qsw[:], rb[:, i, :])
                nc.gpsimd.tensor_add(qt_sb[:, i, cs], t1[:], t2[:])

                t3 = tmppool.tile([128, 512], BF16)
                nc.vector.tensor_mul(t3[:], kraw[:], ra[:, i, :])
                t4 = tmppool.tile([128, 512], BF16)
                nc.vector.tensor_mul(t4[:], ksw[:], rb[:, i, :])
                nc.gpsimd.tensor_add(kt_sb[:, i, cs], t3[:], t4[:])

    # ---------------- phase 2: attention ----------------
    with tc.tile_pool(name="ptp", bufs=3) as ptppool, \
         tc.tile_pool(name="ptd", bufs=3) as ptdpool, \
         tc.tile_pool(name="lnv", bufs=2) as lnpool, \
         tc.tile_pool(name="rcp", bufs=2) as rcppool, \
         tc.tile_pool(name="bcs", bufs=2) as bcspool, \
         tc.tile_pool(name="ovs", bufs=3) as ovspool, \
         tc.tile_pool(name="st_ps", bufs=3, space="PSUM") as stpool, \
         tc.tile_pool(name="ov_ps", bufs=2, space="PSUM") as ovpool, \
         tc.tile_pool(name="nrm_ps", bufs=3, space="PSUM") as nrmpool:

        # Deferred normalization: stage A (evacuate ov, ln of sums, then
        # exp(-ln) -> 1/sum, all off the PE) one iteration later; stage B
        # (PE broadcast + final at_sb multiply) three iterations later.
        stage_a, stage_b = [], []

        def emit_stage_a():
            if stage_a:
                stage_a.pop(0)()

        def emit_stage_b(min_pending):
            while len(stage_b) > min_pending:
                stage_b.pop(0)()

        for qc in range(N_SC):
            for i in range(HPC):
                nkb = 4 * (qc + 1)
                npair = (4 * qc) // 2  # off-diagonal key-block pairs
                nunit = npair + 4
                ov_ps = ovpool.tile([128, 512], F32)
                sum_ps = nrmpool.tile([16, 512], F32, name="nrm_t")

                emit_stage_a()

                def emit_pair(m, pt2, ov_ps=ov_ps, sum_ps=sum_ps):
                    st = m == 0
                    nc.tensor.matmul(ov_ps[:], v_f8[:, m, :, :],
                                     pt2[:], perf_mode=DR,
                                     start=st, stop=False)
                    nc.tensor.matmul(sum_ps[:], ones2[:],
                                     pt2[:], perf_mode=DR,
                                     start=st, stop=False)

                def emit_diag(kb, ptd, qc=qc, ov_ps=ov_ps, sum_ps=sum_ps,
                              nkb=nkb):
                    o = kb - 4 * qc
                    qo = 128 * o
                    n = 512 - qo
                    st = kb == 0
                    sp = kb == nkb - 1
                    nc.tensor.matmul(ov_ps[:, qo:], v_sb[:, kb, :],
                                     ptd[:, :n], start=st, stop=sp)
                    nc.tensor.matmul(sum_ps[0:1, qo:], onesk,
                                     ptd[:, :n], start=st, stop=sp)

                prev = None  # closure awaiting PV/SUM emission
                for u in range(nunit):
                    if u < npair:
                        pt2 = ptppool.tile([128, 2, 512], FP8)
                        for j in range(2):
                            kb = 2 * u + j
                            st_ps = stpool.tile([128, 512], F32)
                            nc.tensor.matmul(
                                st_ps[:],
                                kt_sb[:, i, 128 * kb:128 * (kb + 1)],
                                qt_sb[:, i, 512 * qc:512 * (qc + 1)],
                                start=True, stop=True)
                            nc.scalar.activation(
                                pt2[:, j, :], st_ps[:], EXP,
                                bias=zbias[:], scale=INV_SQRT_D)
                        cur = (lambda u=u, pt2=pt2:
                               emit_pair(u, pt2))
                    else:
                        kb = 4 * qc + (u - npair)
                        o = u - npair
                        qo = 128 * o
                        n = 512 - qo
                        st_ps = stpool.tile([128, 512], F32)
                        nc.tensor.matmul(
                            st_ps[:, :n],
                            kt_sb[:, i, 128 * kb:128 * (kb + 1)],
                            qt_sb[:, i, 512 * qc + qo:512 * (qc + 1)],
                            start=True, stop=False)
                        nc.tensor.matmul(st_ps[:, :n], ident,
                                         mask1[:, :n], start=False, stop=True)
                        ptd = ptdpool.tile([128, 512], BF16)
                        nc.scalar.activation(
                            ptd[:, :n], st_ps[:, :n], EXP,
                            bias=zbias[:], scale=INV_SQRT_D)
                        cur = (lambda kb=kb, ptd=ptd:
                               emit_diag(kb, ptd))
                    if u == 1:
                        emit_stage_b(2)
                    if prev is not None:
                        prev()
                    prev = cur
                prev()

                def a_step(i=i, qc=qc, ov_ps=ov_ps, sum_ps=sum_ps):
                    ovS = ovspool.tile([128, 512], BF16)
                    nc.vector.tensor_copy(ovS[:], ov_ps[:])
                    sumS = lnpool.tile([1, 512], F32)
                    nc.vector.tensor_copy(sumS[:], sum_ps[0:1, :])
                    rcp = rcppool.tile([1, 512], F32R)
                    with nc.allow_low_precision(
                            reason="f32r view of fp32 for matmul rhs"):
                        nc.vector.reciprocal(rcp[:], sumS[:])

                    def b_step(i=i, qc=qc, ovS=ovS, rcp=rcp):
                        bc_ps = nrmpool.tile([128, 512], F32, name="nrm_t")
                        nc.tensor.matmul(bc_ps[:], ones1[:], rcp[:],
                                         start=True, stop=True)
                        bcS = bcspool.tile([128, 512], BF16)
                        nc.vector.tensor_copy(bcS[:], bc_ps[:])
                        nc.vector.tensor_mul(
                            at_sb[:, i, 512 * qc:512 * (qc + 1)],
                            ovS[:], bcS[:])

                    stage_b.append(b_step)

                stage_a.append(a_step)

        emit_stage_a()
        emit_stage_b(0)

    # ---------------- phase 3: output projection ----------------
    with tc.tile_pool(name="osb", bufs=3) as opool, \
         tc.tile_pool(name="op_ps", bufs=3, space="PSUM") as oppool:
        for sc in range(N_SC):
            ss = slice(512 * sc, 512 * (sc + 1))
            for jb in range(N_SB):
                op_ps = oppool.tile([128, 512], F32)
                for h in range(HPC):
                    nc.tensor.matmul(
                        op_ps[:], wo_sb[:, h, 128 * jb:128 * (jb + 1)],
                        at_sb[:, h, ss],
                        start=(h == 0), stop=(h == HPC - 1))
                osb = opool.tile([128, 512], BF16)
                if jb % 2 == 0:
                    nc.scalar.copy(osb[:], op_ps[:])
                else:
                    nc.vector.tensor_copy(osb[:], op_ps[:])
                nc.sync.dma_start(outd[:, jb, sc, :], osb[:])


_NC_CACHE = None


def get_nc():
    global _NC_CACHE
    if _NC_CACHE is not None:
        return _NC_CACHE
    nc = bacc.Bacc("TRN2", target_bir_lowering=False, debug=False,
                   num_devices=8)
    xr = nc.dram_tensor("xr", [128, N_SC, N_DT, 512], BF16,
                        kind="ExternalInput").ap()
    wqr = nc.dram_tensor("wqr", [128, N_DT, 512], BF16,
                         kind="ExternalInput").ap()
    wkr = nc.dram_tensor("wkr", [128, N_DT, 128], BF16,
                         kind="ExternalInput").ap()
    wvr = nc.dram_tensor("wvr", [128, N_DT, 128], BF16,
                         kind="ExternalInput").ap()
    wor = nc.dram_tensor("wor", [128, HPC, S], BF16,
                         kind="ExternalInput").ap()
    rar = nc.dram_tensor("rar", [128, N_SC, HPC, 512], BF16,
                         kind="ExternalInput").ap()
    rbr = nc.dram_tensor("rbr", [128, N_SC, HPC, 512], BF16,
                         kind="ExternalInput").ap()
    constd = nc.dram_tensor("constd", [128, 769], BF16,
                            kind="ExternalInput").ap()
    ons2d = nc.dram_tensor("ons2d", [128, 2, 16], FP8,
                           kind="ExternalInput").ap()
    ons1d = nc.dram_tensor("ons1d", [1, 128], F32R,
                           kind="ExternalInput").ap()
    outd = nc.dram_tensor("outd", [128, N_SB, N_SC, 512], BF16,
                          kind="ExternalOutput").ap()

    with tile.TileContext(nc) as tc, ExitStack() as ctx:
        build_kernel_body(ctx, tc, outd,
                          (xr, wqr, wkr, wvr, wor, rar, rbr, constd,
                           ons2d, ons1d))
    nc.compile()
    _NC_CACHE = nc
    return nc


def host_inputs(x, Wq, Wk, Wv, Wo):
    """Per-core input dicts (core c = 4*b + g), pre-arranged + cast."""
    x = np.asarray(x, np.float32)
    Wq = np.asarray(Wq, np.float32)
    Wk = np.asarray(Wk, np.float32)
    Wv = np.asarray(Wv, np.float32)
    Wo = np.asarray(Wo, np.float32)

    # rope tables (same freqs layout as the reference)
    freqs = 1.0 / (ROPE_THETA ** (np.arange(0, D, 2, dtype=np.float32) / D))
    ang = np.arange(S, dtype=np.float32)[:, None] * freqs[None, :]  # [S, D/2]
    cos = np.cos(ang).astype(np.float32)
    sin = np.sin(ang).astype(np.float32)
    sgn = np.where(np.arange(128) % 2 == 0, -1.0, 1.0).astype(np.float32)

    # packed constants: psw | ident | onesk | mask1
    pswap = np.zeros((128, 128), np.float32)
    idx = np.arange(128)
    pswap[idx, idx ^ 1] = 1.0
    p = np.arange(128)[:, None]
    f = np.arange(512)[None, :]
    mask1 = np.where(p > f, np.float32(NEG), np.float32(0.0))
    constd = np.concatenate(
        [pswap, np.eye(128, dtype=np.float32),
         np.ones((128, 1), np.float32), mask1], axis=1).astype(BF)

    xrs = [np.ascontiguousarray(
        x[b].reshape(N_SC, 512, N_DT, 128).transpose(3, 0, 2, 1)).astype(BF)
        for b in range(B)]

    in_maps = []
    for c in range(8):
        b, g = divmod(c, 4)
        wqr = Wq[512 * g:512 * (g + 1)].reshape(512, N_DT, 128).transpose(
            2, 1, 0).astype(BF)
        wkr = Wk[128 * g:128 * (g + 1)].reshape(128, N_DT, 128).transpose(
            2, 1, 0).astype(BF)
        wvr = Wv[128 * g:128 * (g + 1)].reshape(128, N_DT, 128).transpose(
            2, 1, 0).astype(BF)
        wor = Wo[:, 512 * g:512 * (g + 1)].reshape(S, HPC, 128).transpose(
            2, 1, 0).astype(BF)
        rar = np.empty((128, N_SC, HPC, 512), np.float32)
        rbr = np.empty((128, N_SC, HPC, 512), np.float32)
        for i in range(HPC):
            fidx = 256 * g + 64 * i + (np.arange(128) // 2)  # [128]
            band_a = cos[:, fidx].T  # [128, S]
            band_b = sin[:, fidx].T * sgn[:, None]
            rar[:, :, i, :] = band_a.reshape(128, N_SC, 512)
            rbr[:, :, i, :] = band_b.reshape(128, N_SC, 512)
        in_maps.append({
            "xr": xrs[b],
            "wqr": np.ascontiguousarray(wqr),
            "wkr": np.ascontiguousarray(wkr),
            "wvr": np.ascontiguousarray(wvr),
            "wor": np.ascontiguousarray(wor),
            "rar": rar.astype(BF),
            "rbr": rbr.astype(BF),
            "constd": constd,
            "ons2d": np.ones((128, 2, 16), np.float32).astype(F8),
            "ons1d": np.ones((1, 128), np.float32),
        })
    return in_maps


def kernel(x, Wq, Wk, Wv, Wo, mask, _trace=False):
    in_maps = host_inputs(x, Wq, Wk, Wv, Wo)
    nc = get_nc()
    res = run_bass_kernel_spmd(nc, in_maps, list(range(8)), trace=_trace)
    # outd [128, jb, sc, 512] -> partial [D, S]
    outs = [np.asarray(res.results[c]["outd"], dtype=np.float32)
            .transpose(1, 0, 2, 3).reshape(D, S) for c in range(8)]
    out = np.stack([
        (outs[4 * b + 0] + outs[4 * b + 1] + outs[4 * b + 2]
         + outs[4 * b + 3]).T
        for b in range(B)
    ]).astype(np.float32)
    if _trace:
        kernel.last_result = res
    return out


# revision 11
# speedup vs baseline: 1.2421x; 1.0107x over previous
"""GQA attention kernel for 8 Trainium2 NeuronCores (v3, bf16 + fp8 PV).

Problem: B=2, S=2048, D=2048, 16 q-heads / 4 kv-heads (GQA), head_dim=128,
causal mask, RoPE over the full hidden dim (each head rotates with its own
frequency band), scale 1/sqrt(D), output projection.

Sharding: core c = 4*b + g handles batch b (of 2) and head-group g (of 4):
q-heads 4g..4g+3, which all share kv-head g.  The only cross-core reduction
is the output projection, summed on the host over the 4 head-groups.

v3 changes vs v2 (350us):
  - startup: one packed constants DMA; x chunk-0 split into graded pieces
    interleaved with the wq halves on the sync queue; wo load deferred to
    chunk 1; rope chunk 0 on the gpsimd queue.  First matmul ~9us, not 27.
  - off-diagonal PV + probs-sum matmuls run in fp8e4 with
    perf_mode=DoubleRow (two key-blocks contracted per matmul, 2x rate);
    the exp writes those probs tiles directly in fp8.  Diagonal blocks stay
    bf16 with trimmed widths.  PV and the sum share the same fp8 probs, so
    the normalization stays consistent.
  - causal mask add back on DVE (PE is the bottleneck); the softmax
    reciprocal moved to the scalar engine as exp(-ln(sum)) (both functions
    live in one activation table set), so no multi-us op ever sits in the
    DVE FIFO ahead of the mask adds.
"""

import sys

sys.path.insert(0, "/opt/trn_rl_repo")

from contextlib import ExitStack

import ml_dtypes
import numpy as np

import concourse.bass as bass
import concourse.tile as tile
from concourse import bacc, mybir
from concourse.bass_utils import run_bass_kernel_spmd

B, S, D = 2, 2048, 2048
NH, NG = 16, 4
KVH = NH // NG  # 4
HD = D // NH  # 128
HPC = 4  # q heads per core
ROPE_THETA = 10000.0
INV_SQRT_D = 1.0 / float(np.sqrt(np.float32(D)))
NEG = -1.0e30

F32 = mybir.dt.float32
F32R = mybir.dt.float32r
BF16 = mybir.dt.bfloat16
FP8 = mybir.dt.float8e4
BF = ml_dtypes.bfloat16
F8 = ml_dtypes.float8_e4m3

N_DT = D // 128  # 16 contraction tiles
N_SC = S // 512  # 4 seq chunks of 512
N_SB = S // 128  # 16 seq blocks of 128

EXP = mybir.ActivationFunctionType.Exp
LN = mybir.ActivationFunctionType.Ln
DR = mybir.MatmulPerfMode.DoubleRow


def build_kernel_body(ctx: ExitStack, tc: tile.TileContext, outd, ins):
    nc = tc.nc
    xr, wqr, wkr, wvr, wor, rar, rbr, constd, ons2d, ons1d = ins

    # ---------------- persistent tiles + early DMAs ----------------
    persist = ctx.enter_context(tc.tile_pool(name="persist", bufs=1))
    qt_sb = persist.tile([128, HPC, S], BF16)  # Q^T roped, per head
    kt_sb = persist.tile([128, HPC, S], BF16)  # K^T roped, per band
    v_sb = persist.tile([128, N_SB, 128], BF16)  # V s-major (diag blocks)
    v_f8 = persist.tile([128, N_SB // 2, 2, 128], FP8)  # V pairs (off-diag)
    at_sb = persist.tile([128, HPC, S], BF16)  # attn^T per head
    wo_sb = persist.tile([128, HPC, S], BF16)
    const_sb = persist.tile([128, 769], BF16)  # psw|ident|onesk|mask1
    zbias = persist.tile([128, 1], F32)
    ones2 = persist.tile([128, 2, 16], FP8)
    ones1 = persist.tile([1, 128], F32R)

    psw_sb = const_sb[:, 0:128]
    ident = const_sb[:, 128:256]
    onesk = const_sb[:, 256:257]
    mask1 = const_sb[:, 257:769]

    nc.gpsimd.memset(zbias[:], 0.0)
    nc.gpsimd.dma_start(const_sb[:], constd[:])
    nc.gpsimd.dma_start(ones2[:], ons2d[:])
    nc.gpsimd.dma_start(ones1[:], ons1d[:])

    # ---------------- phase 1: projections + RoPE ----------------
    with tc.tile_pool(name="proj_w", bufs=1) as wpool, \
         tc.tile_pool(name="xc", bufs=2) as xcpool, \
         tc.tile_pool(name="ra", bufs=2) as rapool, \
         tc.tile_pool(name="rb", bufs=2) as rbpool, \
         tc.tile_pool(name="kev", bufs=2) as kevpool, \
         tc.tile_pool(name="vev", bufs=2) as vevpool, \
         tc.tile_pool(name="qev", bufs=3) as qevpool, \
         tc.tile_pool(name="sws", bufs=3) as swspool, \
         tc.tile_pool(name="tmp", bufs=4) as tmppool, \
         tc.tile_pool(name="pacc_kv", bufs=1, space="PSUM") as pkv, \
         tc.tile_pool(name="pacc_q", bufs=1, space="PSUM") as pq, \
         tc.tile_pool(name="pswp", bufs=1, space="PSUM") as pswp_pool, \
         tc.tile_pool(name="pswq", bufs=1, space="PSUM") as pswq_pool:

        wk_sb = wpool.tile([128, N_DT, 128], BF16)
        wv_sb = wpool.tile([128, N_DT, 128], BF16)
        wq_sb = wpool.tile([128, N_DT, 512], BF16)
        nc.sync.dma_start(wk_sb[:], wkr[:])
        nc.scalar.dma_start(wv_sb[:], wvr[:])

        # chunk-0 x tiles in graded pieces, wq halves interleaved
        xcs = []
        xc0 = xcpool.tile([128, N_DT, 512], BF16, name="xc_t")
        nc.sync.dma_start(xc0[:, 0:2, :], xr[:, 0, 0:2, :])
        nc.sync.dma_start(xc0[:, 2:6, :], xr[:, 0, 2:6, :])
        nc.sync.dma_start(wq_sb[:, 0:8, :], wqr[:, 0:8, :])
        nc.sync.dma_start(xc0[:, 6:12, :], xr[:, 0, 6:12, :])
        nc.sync.dma_start(wq_sb[:, 8:16, :], wqr[:, 8:16, :])
        nc.sync.dma_start(xc0[:, 12:16, :], xr[:, 0, 12:16, :])
        xcs.append(xc0)
        ra0 = rapool.tile([128, HPC, 512], BF16, name="ra_t")
        rb0 = rbpool.tile([128, HPC, 512], BF16, name="rb_t")
        nc.gpsimd.dma_start(ra0[:], rar[:, 0])
        nc.gpsimd.dma_start(rb0[:], rbr[:, 0])
        ras, rbs = [ra0], [rb0]

        for c in range(N_SC):
            cs = slice(512 * c, 512 * (c + 1))
            xc = xcs[c]
            if c + 1 < N_SC:
                xcn = xcpool.tile([128, N_DT, 512], BF16, name="xc_t")
                nc.sync.dma_start(xcn[:], xr[:, c + 1])
                xcs.append(xcn)
            ra, rb = ras[c], rbs[c]

            # K/V pass (2 psum banks)
            kv_ps = pkv.tile([128, 2, 512], F32)
            for dt in range(N_DT):
                st, sp = dt == 0, dt == N_DT - 1
                nc.tensor.matmul(kv_ps[:, 0, :], wk_sb[:, dt, :],
                                 xc[:, dt, :], start=st, stop=sp)
                nc.tensor.matmul(kv_ps[:, 1, :], wv_sb[:, dt, :],
                                 xc[:, dt, :], start=st, stop=sp)
            kraw = kevpool.tile([128, 512], BF16)
            nc.scalar.copy(kraw[:], kv_ps[:, 0, :])
            vtr = vevpool.tile([128, 512], BF16)
            nc.scalar.copy(vtr[:], kv_ps[:, 1, :])
            if c + 1 < N_SC:
                ran = rapool.tile([128, HPC, 512], BF16, name="ra_t")
                rbn = rbpool.tile([128, HPC, 512], BF16, name="rb_t")
                nc.scalar.dma_start(ran[:], rar[:, c + 1])
                nc.scalar.dma_start(rbn[:], rbr[:, c + 1])
                ras.append(ran)
                rbs.append(rbn)
            if c == 1:
                nc.scalar.dma_start(wo_sb[:], wor[:])

            # Q pass (4 psum banks)
            q_ps = pq.tile([128, HPC, 512], F32)
            for dt in range(N_DT):
                st, sp = dt == 0, dt == N_DT - 1
                for i in range(HPC):
                    nc.tensor.matmul(
                        q_ps[:, i, :], wq_sb[:, dt, 128 * i:128 * (i + 1)],
                        xc[:, dt, :], start=st, stop=sp)

            # V: transpose hd-major -> s-major; dual bf16 + fp8 copies
            for j in range(4):
                kb = 4 * c + j
                vt_ps = pswq_pool.tile([128, 128], BF16, name="swq_t")
                nc.tensor.transpose(vt_ps[:],
                                    vtr[:, 128 * j:128 * (j + 1)], ident)
                nc.scalar.copy(v_sb[:, kb, :], vt_ps[:])
                nc.scalar.copy(v_f8[:, kb // 2, kb % 2, :], vt_ps[:])

            # K swap (pair-exchange along partitions) via permutation matmul
            ksw_ps = pswp_pool.tile([128, 512], F32)
            nc.tensor.matmul(ksw_ps[:], psw_sb, kraw[:],
                             start=True, stop=True)
            ksw = swspool.tile([128, 512], BF16, name="sw_t")
            nc.scalar.copy(ksw[:], ksw_ps[:])

            # RoPE per head/band
            for i in range(HPC):
                qraw = qevpool.tile([128, 512], BF16)
                nc.scalar.copy(qraw[:], q_ps[:, i, :])
                qsw_ps = pswq_pool.tile([128, 512], F32, name="swq_t")
                nc.tensor.matmul(qsw_ps[:], psw_sb, qraw[:],
                                 start=True, stop=True)
                qsw = swspool.tile([128, 512], BF16, name="sw_t")
                nc.scalar.copy(qsw[:], qsw_ps[:])

                t1 = tmppool.tile([128, 512], BF16)
                nc.vector.tensor_mul(t1[:], qraw[:], ra[:, i, :])
                t2 = tmppool.tile([128, 512], BF16)
                nc.vector.tensor_mul(t2[:], qsw[:], rb[:, i, :])
                nc.gpsimd.tensor_add(qt_sb[:, i, cs], t1[:], t2[:])

                t3 = tmppool.tile([128, 512], BF16)
                nc.vector.tensor_mul(t3[:], kraw[:], ra[:, i, :])
                t4 = tmppool.tile([128, 512], BF16)
                nc.vector.tensor_mul(t4[:], ksw[:], rb[:, i, :])
                nc.gpsimd.tensor_add(kt_sb[:, i, cs], t3[:], t4[:])

    # ---------------- phase 2: attention ----------------
    with tc.tile_pool(name="ptp", bufs=3) as ptppool, \
         tc.tile_pool(name="ptd", bufs=3) as ptdpool, \
         tc.tile_pool(name="lnv", bufs=2) as lnpool, \
         tc.tile_pool(name="rcp", bufs=2) as rcppool, \
         tc.tile_pool(name="bcs", bufs=2) as bcspool, \
         tc.tile_pool(name="ovs", bufs=3) as ovspool, \
         tc.tile_pool(name="st_ps", bufs=3, space="PSUM") as stpool, \
         tc.tile_pool(name="ov_ps", bufs=2, space="PSUM") as ovpool, \
         tc.tile_pool(name="nrm_ps", bufs=3, space="PSUM") as nrmpool:

        # Deferred normalization: stage A (evacuate ov, ln of sums, then
        # exp(-ln) -> 1/sum, all off the PE) one iteration later; stage B
        # (PE broadcast + final at_sb multiply) three iterations later.
        stage_a, stage_b = [], []

        def emit_stage_a():
            if stage_a:
                stage_a.pop(0)()

        def emit_stage_b(min_pending):
            while len(stage_b) > min_pending:
                stage_b.pop(0)()

        for qc in range(N_SC):
            for i in range(HPC):
                nkb = 4 * (qc + 1)
                npair = (4 * qc) // 2  # off-diagonal key-block pairs
                nunit = npair + 4
                ov_ps = ovpool.tile([128, 512], F32)
                sum_ps = nrmpool.tile([16, 512], F32, name="nrm_t")

                emit_stage_a()

                def emit_pair(m, pt2, ov_ps=ov_ps, sum_ps=sum_ps):
                    st = m == 0
                    nc.tensor.matmul(ov_ps[:], v_f8[:, m, :, :],
                                     pt2[:], perf_mode=DR,
                                     start=st, stop=False)
                    nc.tensor.matmul(sum_ps[:], ones2[:],
                                     pt2[:], perf_mode=DR,
                                     start=st, stop=False)

                def emit_diag(kb, ptd, qc=qc, ov_ps=ov_ps, sum_ps=sum_ps,
                              nkb=nkb):
                    o = kb - 4 * qc
                    qo = 128 * o
                    n = 512 - qo
                    st = kb == 0
                    sp = kb == nkb - 1
                    nc.tensor.matmul(ov_ps[:, qo:], v_sb[:, kb, :],
                                     ptd[:, :n], start=st, stop=sp)
                    nc.tensor.matmul(sum_ps[0:1, qo:], onesk,
                                     ptd[:, :n], start=st, stop=sp)

                prev = None  # closure awaiting PV/SUM emission
                for u in range(nunit):
                    if u < npair:
                        pt2 = ptppool.tile([128, 2, 512], FP8)
                        for j in range(2):
                            kb = 2 * u + j
                            st_ps = stpool.tile([128, 512], F32)
                            nc.tensor.matmul(
                                st_ps[:],
                                kt_sb[:, i, 128 * kb:128 * (kb + 1)],
                                qt_sb[:, i, 512 * qc:512 * (qc + 1)],
                                start=True, stop=True)
                            nc.scalar.activation(
                                pt2[:, j, :], st_ps[:], EXP,
                                bias=zbias[:], scale=INV_SQRT_D)
                        cur = (lambda u=u, pt2=pt2:
                               emit_pair(u, pt2))
                    else:
                        kb = 4 * qc + (u - npair)
                        o = u - npair
                        qo = 128 * o
                        n = 512 - qo
                        st_ps = stpool.tile([128, 512], F32)
                        nc.tensor.matmul(
                            st_ps[:, :n],
                            kt_sb[:, i, 128 * kb:128 * (kb + 1)],
                            qt_sb[:, i, 512 * qc + qo:512 * (qc + 1)],
                            start=True, stop=False)
                        nc.tensor.matmul(st_ps[:, :n], ident,
                                         mask1[:, :n], start=False, stop=True)
                        ptd = ptdpool.tile([128, 512], BF16)
                        nc.scalar.activation(
                            ptd[:, :n], st_ps[:, :n], EXP,
                            bias=zbias[:], scale=INV_SQRT_D)
                        cur = (lambda kb=kb, ptd=ptd:
                               emit_diag(kb, ptd))
                    if u == 1:
                        emit_stage_b(2)
                    if prev is not None:
                        prev()
                    prev = cur
                prev()

                def a_step(i=i, qc=qc, ov_ps=ov_ps, sum_ps=sum_ps):
                    ovS = ovspool.tile([128, 512], BF16)
                    nc.vector.tensor_copy(ovS[:], ov_ps[:])
                    sumS = lnpool.tile([1, 512], F32)
                    nc.vector.tensor_copy(sumS[:], sum_ps[0:1, :])
                    rcp = rcppool.tile([1, 512], F32R)
                    with nc.allow_low_precision(
                            reason="f32r view of fp32 for matmul rhs"):
                        nc.vector.reciprocal(rcp[:], sumS[:])

                    def b_step(i=i, qc=qc, ovS=ovS, rcp=rcp):
                        bc_ps = nrmpool.tile([128, 512], F32, name="nrm_t")
                        nc.tensor.matmul(bc_ps[:], ones1[:], rcp[:],
                                         start=True, stop=True)
                        bcS = bcspool.tile([128, 512], BF16)
                        nc.vector.tensor_copy(bcS[:], bc_ps[:])
                        nc.vector.tensor_mul(
                            at_sb[:, i, 512 * qc:512 * (qc + 1)],
                            ovS[:], bcS[:])

                    stage_b.append(b_step)

                stage_a.append(a_step)

        emit_stage_a()
        emit_stage_b(0)

    # ---------------- phase 3: output projection ----------------
    with tc.tile_pool(name="osb", bufs=3) as opool, \
         tc.tile_pool(name="op_ps", bufs=3, space="PSUM") as oppool:
        for sc in range(N_SC):
            ss = slice(512 * sc, 512 * (sc + 1))
            for jb in range(N_SB):
                op_ps = oppool.tile([128, 512], F32)
                for h in range(HPC):
                    nc.tensor.matmul(
                        op_ps[:], wo_sb[:, h, 128 * jb:128 * (jb + 1)],
                        at_sb[:, h, ss],
                        start=(h == 0), stop=(h == HPC - 1))
                osb = opool.tile([128, 512], BF16)
                if jb % 2 == 0:
                    nc.scalar.copy(osb[:], op_ps[:])
                else:
                    nc.vector.tensor_copy(osb[:], op_ps[:])
                nc.sync.dma_start(outd[:, jb, sc, :], osb[:])


_NC_CACHE = None


def get_nc():
    global _NC_CACHE
    if _NC_CACHE is not None:
        return _NC_CACHE
    nc = bacc.Bacc("TRN2", target_bir_lowering=False, debug=False,
                   num_devices=8)
    xr = nc.dram_tensor("xr", [128, N_SC, N_DT, 512], BF16,
                        kind="ExternalInput").ap()
    wqr = nc.dram_tensor("wqr", [128, N_DT, 512], BF16,
                         kind="ExternalInput").ap()
    wkr = nc.dram_tensor("wkr", [128, N_DT, 128], BF16,
                         kind="ExternalInput").ap()
    wvr = nc.dram_tensor("wvr", [128, N_DT, 128], BF16,
                         kind="ExternalInput").ap()
    wor = nc.dram_tensor("wor", [128, HPC, S], BF16,
                         kind="ExternalInput").ap()
    rar = nc.dram_tensor("rar", [128, N_SC, HPC, 512], BF16,
                         kind="ExternalInput").ap()
    rbr = nc.dram_tensor("rbr", [128, N_SC, HPC, 512], BF16,
                         kind="ExternalInput").ap()
    constd = nc.dram_tensor("constd", [128, 769], BF16,
                            kind="ExternalInput").ap()
    ons2d = nc.dram_tensor("ons2d", [128, 2, 16], FP8,
                           kind="ExternalInput").ap()
    ons1d = nc.dram_tensor("ons1d", [1, 128], F32R,
                           kind="ExternalInput").ap()
    outd = nc.dram_tensor("outd", [128, N_SB, N_SC, 512], BF16,
                          kind="ExternalOutput").ap()

    with tile.TileContext(nc) as tc, ExitStack() as ctx:
        build_kernel_body(ctx, tc, outd,
                          (xr, wqr, wkr, wvr, wor, rar, rbr, constd,
                           ons2d, ons1d))
    nc.compile()
    _NC_CACHE = nc
    return nc


def host_inputs(x, Wq, Wk, Wv, Wo):
    """Per-core input dicts (core c = 4*b + g), pre-arranged + cast."""
    x = np.asarray(x, np.float32)
    Wq = np.asarray(Wq, np.float32)
    Wk = np.asarray(Wk, np.float32)
    Wv = np.asarray(Wv, np.float32)
    Wo = np.asarray(Wo, np.float32)

    # rope tables (same freqs layout as the reference)
    freqs = 1.0 / (ROPE_THETA ** (np.arange(0, D, 2, dtype=np.float32) / D))
    ang = np.arange(S, dtype=np.float32)[:, None] * freqs[None, :]  # [S, D/2]
    cos = np.cos(ang).astype(np.float32)
    sin = np.sin(ang).astype(np.float32)
    sgn = np.where(np.arange(128) % 2 == 0, -1.0, 1.0).astype(np.float32)

    # packed constants: psw | ident | onesk | mask1
    pswap = np.zeros((128, 128), np.float32)
    idx = np.arange(128)
    pswap[idx, idx ^ 1] = 1.0
    p = np.arange(128)[:, None]
    f = np.arange(512)[None, :]
    mask1 = np.where(p > f, np.float32(NEG), np.float32(0.0))
    constd = np.concatenate(
        [pswap, np.eye(128, dtype=np.float32),
         np.ones((128, 1), np.float32), mask1], axis=1).astype(BF)

    xrs = [np.ascontiguousarray(
        x[b].reshape(N_SC, 512, N_DT, 128).transpose(3, 0, 2, 1)).astype(BF)
        for b in range(B)]

    in_maps = []
    for c in range(8):
        b, g = divmod(c, 4)
        wqr = Wq[512 * g:512 * (g + 1)].reshape(512, N_DT, 128).transpose(
            2, 1, 0).astype(BF)
        wkr = Wk[128 * g:128 * (g + 1)].reshape(128, N_DT, 128).transpose(
            2, 1, 0).astype(BF)
        wvr = Wv[128 * g:128 * (g + 1)].reshape(128, N_DT, 128).transpose(
            2, 1, 0).astype(BF)
        wor = Wo[:, 512 * g:512 * (g + 1)].reshape(S, HPC, 128).transpose(
            2, 1, 0).astype(BF)
        rar = np.empty((128, N_SC, HPC, 512), np.float32)
        rbr = np.empty((128, N_SC, HPC, 512), np.float32)
        for i in range(HPC):
            fidx = 256 * g + 64 * i + (np.arange(128) // 2)  # [128]
            band_a = cos[:, fidx].T  # [128, S]
            band_b = sin[:, fidx].T * sgn[:, None]
            rar[:, :, i, :] = band_a.reshape(128, N_SC, 512)
            rbr[:, :, i, :] = band_b.reshape(128, N_SC, 512)
        in_maps.append({
            "xr": xrs[b],
            "wqr": np.ascontiguousarray(wqr),
            "wkr": np.ascontiguousarray(wkr),
            "wvr": np.ascontiguousarray(wvr),
            "wor": np.ascontiguousarray(wor),
            "rar": rar.astype(BF),
            "rbr": rbr.astype(BF),
            "constd": constd,
            "ons2d": np.ones((128, 2, 16), np.float32).astype(F8),
            "ons1d": np.ones((1, 128), np.float32),
        })
    return in_maps


def kernel(x, Wq, Wk, Wv, Wo, mask, _trace=False):
    in_maps = host_inputs(x, Wq, Wk, Wv, Wo)
    nc = get_nc()
    res = run_bass_kernel_spmd(nc, in_maps, list(range(8)), trace=_trace)
    # outd [128, jb, sc, 512] -> partial [D, S]
    outs = [np.asarray(res.results[c]["outd"], dtype=np.float32)
            .transpose(1, 0, 2, 3).reshape(D, S) for c in range(8)]
    out = np.stack([
        (outs[4 * b + 0] + outs[4 * b + 1] + outs[4 * b + 2]
         + outs[4 * b + 3]).T
        for b in range(B)
    ]).astype(np.float32)
    if _trace:
        kernel.last_result = res
    return out


# revision 13
# speedup vs baseline: 1.2763x; 1.0276x over previous
"""GQA attention kernel for 8 Trainium2 NeuronCores (v3, bf16 + fp8 PV).

Problem: B=2, S=2048, D=2048, 16 q-heads / 4 kv-heads (GQA), head_dim=128,
causal mask, RoPE over the full hidden dim (each head rotates with its own
frequency band), scale 1/sqrt(D), output projection.

Sharding: core c = 4*b + g handles batch b (of 2) and head-group g (of 4):
q-heads 4g..4g+3, which all share kv-head g.  The only cross-core reduction
is the output projection, summed on the host over the 4 head-groups.

v3 changes vs v2 (350us):
  - startup: one packed constants DMA; x chunk-0 split into graded pieces
    interleaved with the wq halves on the sync queue; wo load deferred to
    chunk 1; rope chunk 0 on the gpsimd queue.  First matmul ~9us, not 27.
  - off-diagonal PV + probs-sum matmuls run in fp8e4 with
    perf_mode=DoubleRow (two key-blocks contracted per matmul, 2x rate);
    the exp writes those probs tiles directly in fp8.  Diagonal blocks stay
    bf16 with trimmed widths.  PV and the sum share the same fp8 probs, so
    the normalization stays consistent.
  - causal mask add back on DVE (PE is the bottleneck); the softmax
    reciprocal moved to the scalar engine as exp(-ln(sum)) (both functions
    live in one activation table set), so no multi-us op ever sits in the
    DVE FIFO ahead of the mask adds.
"""

import sys

sys.path.insert(0, "/opt/trn_rl_repo")

from contextlib import ExitStack

import ml_dtypes
import numpy as np

import concourse.bass as bass
import concourse.tile as tile
from concourse.tile import add_dep_helper
from concourse import bacc, mybir
from concourse.bass_utils import run_bass_kernel_spmd

B, S, D = 2, 2048, 2048
NH, NG = 16, 4
KVH = NH // NG  # 4
HD = D // NH  # 128
HPC = 4  # q heads per core
ROPE_THETA = 10000.0
INV_SQRT_D = 1.0 / float(np.sqrt(np.float32(D)))
NEG = -1.0e30

F32 = mybir.dt.float32
F32R = mybir.dt.float32r
BF16 = mybir.dt.bfloat16
FP8 = mybir.dt.float8e4
BF = ml_dtypes.bfloat16
F8 = ml_dtypes.float8_e4m3

N_DT = D // 128  # 16 contraction tiles
N_SC = S // 512  # 4 seq chunks of 512
N_SB = S // 128  # 16 seq blocks of 128

EXP = mybir.ActivationFunctionType.Exp
LN = mybir.ActivationFunctionType.Ln
DR = mybir.MatmulPerfMode.DoubleRow


def build_kernel_body(ctx: ExitStack, tc: tile.TileContext, outd, ins):
    nc = tc.nc
    xr, wqr, wkr, wvr, wor, rar, rbr, constd, ons2d, ons1d = ins

    # ---------------- persistent tiles + early DMAs ----------------
    persist = ctx.enter_context(tc.tile_pool(name="persist", bufs=1))
    qt_sb = persist.tile([128, HPC, S], BF16)  # Q^T roped, per head
    kt_sb = persist.tile([128, HPC, S], BF16)  # K^T roped, per band
    v_sb = persist.tile([128, N_SB, 128], BF16)  # V s-major (diag blocks)
    v_f8 = persist.tile([128, N_SB // 2, 2, 128], FP8)  # V pairs (off-diag)
    at_sb = persist.tile([128, HPC, S], BF16)  # attn^T per head
    wo_sb = persist.tile([128, HPC, S], BF16)
    const_sb = persist.tile([128, 769], BF16)  # psw|ident|onesk|mask1
    zbias = persist.tile([128, 1], F32)
    ones2 = persist.tile([128, 2, 16], FP8)
    ones1 = persist.tile([1, 128], F32R)

    psw_sb = const_sb[:, 0:128]
    ident = const_sb[:, 128:256]
    onesk = const_sb[:, 256:257]
    mask1 = const_sb[:, 257:769]

    nc.gpsimd.memset(zbias[:], 0.0)
    nc.gpsimd.dma_start(const_sb[:], constd[:])
    nc.gpsimd.dma_start(ones2[:], ons2d[:])
    nc.gpsimd.dma_start(ones1[:], ons1d[:])

    # ---------------- phase 1: projections + RoPE ----------------
    with tc.tile_pool(name="proj_w", bufs=1) as wpool, \
         tc.tile_pool(name="xc", bufs=2) as xcpool, \
         tc.tile_pool(name="ra", bufs=2) as rapool, \
         tc.tile_pool(name="rb", bufs=2) as rbpool, \
         tc.tile_pool(name="kev", bufs=2) as kevpool, \
         tc.tile_pool(name="vev", bufs=2) as vevpool, \
         tc.tile_pool(name="qev", bufs=3) as qevpool, \
         tc.tile_pool(name="sws", bufs=3) as swspool, \
         tc.tile_pool(name="tmp", bufs=4) as tmppool, \
         tc.tile_pool(name="pacc_kv", bufs=1, space="PSUM") as pkv, \
         tc.tile_pool(name="pacc_q", bufs=1, space="PSUM") as pq, \
         tc.tile_pool(name="pswp", bufs=1, space="PSUM") as pswp_pool, \
         tc.tile_pool(name="pswq", bufs=1, space="PSUM") as pswq_pool:

        wk_sb = wpool.tile([128, N_DT, 128], BF16)
        wv_sb = wpool.tile([128, N_DT, 128], BF16)
        wq_sb = wpool.tile([128, N_DT, 512], BF16)
        nc.sync.dma_start(wk_sb[:], wkr[:])
        nc.scalar.dma_start(wv_sb[:], wvr[:])

        # chunk-0 x tiles in graded pieces, wq halves interleaved
        xcs = []
        xc0 = xcpool.tile([128, N_DT, 512], BF16, name="xc_t")
        nc.sync.dma_start(xc0[:, 0:2, :], xr[:, 0, 0:2, :])
        nc.sync.dma_start(xc0[:, 2:6, :], xr[:, 0, 2:6, :])
        nc.sync.dma_start(wq_sb[:, 0:8, :], wqr[:, 0:8, :])
        nc.sync.dma_start(xc0[:, 6:12, :], xr[:, 0, 6:12, :])
        nc.sync.dma_start(wq_sb[:, 8:16, :], wqr[:, 8:16, :])
        nc.sync.dma_start(xc0[:, 12:16, :], xr[:, 0, 12:16, :])
        xcs.append(xc0)
        ra0 = rapool.tile([128, HPC, 512], BF16, name="ra_t")
        rb0 = rbpool.tile([128, HPC, 512], BF16, name="rb_t")
        nc.gpsimd.dma_start(ra0[:], rar[:, 0])
        nc.gpsimd.dma_start(rb0[:], rbr[:, 0])
        ras, rbs = [ra0], [rb0]

        for c in range(N_SC):
            cs = slice(512 * c, 512 * (c + 1))
            xc = xcs[c]
            if c + 1 < N_SC:
                xcn = xcpool.tile([128, N_DT, 512], BF16, name="xc_t")
                nc.sync.dma_start(xcn[:], xr[:, c + 1])
                xcs.append(xcn)
            ra, rb = ras[c], rbs[c]

            # K/V pass (2 psum banks)
            kv_ps = pkv.tile([128, 2, 512], F32)
            for dt in range(N_DT):
                st, sp = dt == 0, dt == N_DT - 1
                nc.tensor.matmul(kv_ps[:, 0, :], wk_sb[:, dt, :],
                                 xc[:, dt, :], start=st, stop=sp)
                nc.tensor.matmul(kv_ps[:, 1, :], wv_sb[:, dt, :],
                                 xc[:, dt, :], start=st, stop=sp)
            kraw = kevpool.tile([128, 512], BF16)
            kraw_i = nc.scalar.copy(kraw[:], kv_ps[:, 0, :])
            vtr = vevpool.tile([128, 512], BF16)
            nc.scalar.copy(vtr[:], kv_ps[:, 1, :])
            if c + 1 < N_SC:
                ran = rapool.tile([128, HPC, 512], BF16, name="ra_t")
                rbn = rbpool.tile([128, HPC, 512], BF16, name="rb_t")
                d1 = nc.scalar.dma_start(ran[:], rar[:, c + 1])
                d2 = nc.scalar.dma_start(rbn[:], rbr[:, c + 1])
                add_dep_helper(d1.ins, kraw_i.ins, sync=False,
                               reason="defer rope prefetch issue")
                add_dep_helper(d2.ins, kraw_i.ins, sync=False,
                               reason="defer rope prefetch issue")
                ras.append(ran)
                rbs.append(rbn)
            if c == 1:
                dwo = nc.scalar.dma_start(wo_sb[:], wor[:])
                add_dep_helper(dwo.ins, kraw_i.ins, sync=False,
                               reason="defer wo load issue")

            # Q pass (4 psum banks)
            q_ps = pq.tile([128, HPC, 512], F32)
            for dt in range(N_DT):
                st, sp = dt == 0, dt == N_DT - 1
                for i in range(HPC):
                    nc.tensor.matmul(
                        q_ps[:, i, :], wq_sb[:, dt, 128 * i:128 * (i + 1)],
                        xc[:, dt, :], start=st, stop=sp)

            # V: transpose hd-major -> s-major; dual bf16 + fp8 copies
            for j in range(4):
                kb = 4 * c + j
                vt_ps = pswq_pool.tile([128, 128], BF16, name="swq_t")
                nc.tensor.transpose(vt_ps[:],
                                    vtr[:, 128 * j:128 * (j + 1)], ident)
                nc.scalar.copy(v_sb[:, kb, :], vt_ps[:])
                nc.scalar.copy(v_f8[:, kb // 2, kb % 2, :], vt_ps[:])

            # K swap (pair-exchange along partitions) via permutation matmul
            ksw_ps = pswp_pool.tile([128, 512], F32)
            nc.tensor.matmul(ksw_ps[:], psw_sb, kraw[:],
                             start=True, stop=True)
            ksw = swspool.tile([128, 512], BF16, name="sw_t")
            nc.scalar.copy(ksw[:], ksw_ps[:])

            # RoPE per head/band
            for i in range(HPC):
                qraw = qevpool.tile([128, 512], BF16)
                nc.scalar.copy(qraw[:], q_ps[:, i, :])
                qsw_ps = pswq_pool.tile([128, 512], F32, name="swq_t")
                nc.tensor.matmul(qsw_ps[:], psw_sb, qraw[:],
                                 start=True, stop=True)
                qsw = swspool.tile([128, 512], BF16, name="sw_t")
                nc.scalar.copy(qsw[:], qsw_ps[:])

                t1 = tmppool.tile([128, 512], BF16)
                nc.vector.tensor_mul(t1[:], qraw[:], ra[:, i, :])
                t2 = tmppool.tile([128, 512], BF16)
                nc.vector.tensor_mul(t2[:], qsw[:], rb[:, i, :])
                nc.gpsimd.tensor_add(qt_sb[:, i, cs], t1[:], t2[:])

                t3 = tmppool.tile([128, 512], BF16)
                nc.vector.tensor_mul(t3[:], kraw[:], ra[:, i, :])
                t4 = tmppool.tile([128, 512], BF16)
                nc.vector.tensor_mul(t4[:], ksw[:], rb[:, i, :])
                nc.gpsimd.tensor_add(kt_sb[:, i, cs], t3[:], t4[:])

    # ---------------- phase 2: attention ----------------
    with tc.tile_pool(name="ptp", bufs=3) as ptppool, \
         tc.tile_pool(name="ptd", bufs=3) as ptdpool, \
         tc.tile_pool(name="lnv", bufs=2) as lnpool, \
         tc.tile_pool(name="rcp", bufs=2) as rcppool, \
         tc.tile_pool(name="bcs", bufs=2) as bcspool, \
         tc.tile_pool(name="ovs", bufs=3) as ovspool, \
         tc.tile_pool(name="st_ps", bufs=3, space="PSUM") as stpool, \
         tc.tile_pool(name="ov_ps", bufs=2, space="PSUM") as ovpool, \
         tc.tile_pool(name="nrm_ps", bufs=3, space="PSUM") as nrmpool:

        # Deferred normalization: stage A (evacuate ov, ln of sums, then
        # exp(-ln) -> 1/sum, all off the PE) one iteration later; stage B
        # (PE broadcast + final at_sb multiply) three iterations later.
        stage_a, stage_b = [], []

        def emit_stage_a():
            if stage_a:
                stage_a.pop(0)()

        def emit_stage_b(min_pending):
            while len(stage_b) > min_pending:
                stage_b.pop(0)()

        for qc in range(N_SC):
            for i in range(HPC):
                nkb = 4 * (qc + 1)
                npair = (4 * qc) // 2  # off-diagonal key-block pairs
                nunit = npair + 4
                ov_ps = ovpool.tile([128, 512], F32)
                sum_ps = nrmpool.tile([16, 512], F32, name="nrm_t")

                emit_stage_a()

                def emit_pair(m, pt2, ov_ps=ov_ps, sum_ps=sum_ps):
                    st = m == 0
                    nc.tensor.matmul(ov_ps[:], v_f8[:, m, :, :],
                                     pt2[:], perf_mode=DR,
                                     start=st, stop=False)
                    nc.tensor.matmul(sum_ps[:], ones2[:],
                                     pt2[:], perf_mode=DR,
                                     start=st, stop=False)

                def emit_diag(kb, ptd, qc=qc, ov_ps=ov_ps, sum_ps=sum_ps,
                              nkb=nkb):
                    o = kb - 4 * qc
                    qo = 128 * o
                    n = 512 - qo
                    st = kb == 0
                    sp = kb == nkb - 1
                    nc.tensor.matmul(ov_ps[:, qo:], v_sb[:, kb, :],
                                     ptd[:, :n], start=st, stop=sp)
                    nc.tensor.matmul(sum_ps[0:1, qo:], onesk,
                                     ptd[:, :n], start=st, stop=sp)

                prev = None  # closure awaiting PV/SUM emission
                for u in range(nunit):
                    if u < npair:
                        pt2 = ptppool.tile([128, 2, 512], FP8)
                        for j in range(2):
                            kb = 2 * u + j
                            st_ps = stpool.tile([128, 512], F32)
                            nc.tensor.matmul(
                                st_ps[:],
                                kt_sb[:, i, 128 * kb:128 * (kb + 1)],
                                qt_sb[:, i, 512 * qc:512 * (qc + 1)],
                                start=True, stop=True)
                            nc.scalar.activation(
                                pt2[:, j, :], st_ps[:], EXP,
                                bias=zbias[:], scale=INV_SQRT_D)
                        cur = (lambda u=u, pt2=pt2:
                               emit_pair(u, pt2))
                    else:
                        kb = 4 * qc + (u - npair)
                        o = u - npair
                        qo = 128 * o
                        n = 512 - qo
                        st_ps = stpool.tile([128, 512], F32)
                        nc.tensor.matmul(
                            st_ps[:, :n],
                            kt_sb[:, i, 128 * kb:128 * (kb + 1)],
                            qt_sb[:, i, 512 * qc + qo:512 * (qc + 1)],
                            start=True, stop=False)
                        nc.tensor.matmul(st_ps[:, :n], ident,
                                         mask1[:, :n], start=False, stop=True)
                        ptd = ptdpool.tile([128, 512], BF16)
                        nc.scalar.activation(
                            ptd[:, :n], st_ps[:, :n], EXP,
                            bias=zbias[:], scale=INV_SQRT_D)
                        cur = (lambda kb=kb, ptd=ptd:
                               emit_diag(kb, ptd))
                    if u == 1:
                        emit_stage_b(2)
                    if prev is not None:
                        prev()
                    prev = cur
                prev()

                def a_step(i=i, qc=qc, ov_ps=ov_ps, sum_ps=sum_ps):
                    ovS = ovspool.tile([128, 512], BF16)
                    nc.vector.tensor_copy(ovS[:], ov_ps[:])
                    sumS = lnpool.tile([1, 512], F32)
                    nc.vector.tensor_copy(sumS[:], sum_ps[0:1, :])
                    rcp = rcppool.tile([1, 512], F32R)
                    with nc.allow_low_precision(
                            reason="f32r view of fp32 for matmul rhs"):
                        nc.vector.reciprocal(rcp[:], sumS[:])

                    def b_step(i=i, qc=qc, ovS=ovS, rcp=rcp):
                        bc_ps = nrmpool.tile([128, 512], F32, name="nrm_t")
                        nc.tensor.matmul(bc_ps[:], ones1[:], rcp[:],
                                         start=True, stop=True)
                        bcS = bcspool.tile([128, 512], BF16)
                        nc.vector.tensor_copy(bcS[:], bc_ps[:])
                        nc.vector.tensor_mul(
                            at_sb[:, i, 512 * qc:512 * (qc + 1)],
                            ovS[:], bcS[:])

                    stage_b.append(b_step)

                stage_a.append(a_step)

        emit_stage_a()
        emit_stage_b(0)

    # ---------------- phase 3: output projection ----------------
    with tc.tile_pool(name="osb", bufs=3) as opool, \
         tc.tile_pool(name="op_ps", bufs=3, space="PSUM") as oppool:
        for sc in range(N_SC):
            ss = slice(512 * sc, 512 * (sc + 1))
            for jb in range(N_SB):
                op_ps = oppool.tile([128, 512], F32)
                for h in range(HPC):
                    nc.tensor.matmul(
                        op_ps[:], wo_sb[:, h, 128 * jb:128 * (jb + 1)],
                        at_sb[:, h, ss],
                        start=(h == 0), stop=(h == HPC - 1))
                osb = opool.tile([128, 512], BF16)
                if jb % 2 == 0:
                    nc.scalar.copy(osb[:], op_ps[:])
                else:
                    nc.vector.tensor_copy(osb[:], op_ps[:])
                nc.sync.dma_start(outd[:, jb, sc, :], osb[:])


_NC_CACHE = None


def get_nc():
    global _NC_CACHE
    if _NC_CACHE is not None:
        return _NC_CACHE
    nc = bacc.Bacc("TRN2", target_bir_lowering=False, debug=False,
                   num_devices=8)
    xr = nc.dram_tensor("xr", [128, N_SC, N_DT, 512], BF16,
                        kind="ExternalInput").ap()
    wqr = nc.dram_tensor("wqr", [128, N_DT, 512], BF16,
                         kind="ExternalInput").ap()
    wkr = nc.dram_tensor("wkr", [128, N_DT, 128], BF16,
                         kind="ExternalInput").ap()
    wvr = nc.dram_tensor("wvr", [128, N_DT, 128], BF16,
                         kind="ExternalInput").ap()
    wor = nc.dram_tensor("wor", [128, HPC, S], BF16,
                         kind="ExternalInput").ap()
    rar = nc.dram_tensor("rar", [128, N_SC, HPC, 512], BF16,
                         kind="ExternalInput").ap()
    rbr = nc.dram_tensor("rbr", [128, N_SC, HPC, 512], BF16,
                         kind="ExternalInput").ap()
    constd = nc.dram_tensor("constd", [128, 769], BF16,
                            kind="ExternalInput").ap()
    ons2d = nc.dram_tensor("ons2d", [128, 2, 16], FP8,
                           kind="ExternalInput").ap()
    ons1d = nc.dram_tensor("ons1d", [1, 128], F32R,
                           kind="ExternalInput").ap()
    outd = nc.dram_tensor("outd", [128, N_SB, N_SC, 512], BF16,
                          kind="ExternalOutput").ap()

    with tile.TileContext(nc) as tc, ExitStack() as ctx:
        build_kernel_body(ctx, tc, outd,
                          (xr, wqr, wkr, wvr, wor, rar, rbr, constd,
                           ons2d, ons1d))
    nc.compile()
    _NC_CACHE = nc
    return nc


def host_inputs(x, Wq, Wk, Wv, Wo):
    """Per-core input dicts (core c = 4*b + g), pre-arranged + cast."""
    x = np.asarray(x, np.float32)
    Wq = np.asarray(Wq, np.float32)
    Wk = np.asarray(Wk, np.float32)
    Wv = np.asarray(Wv, np.float32)
    Wo = np.asarray(Wo, np.float32)

    # rope tables (same freqs layout as the reference)
    freqs = 1.0 / (ROPE_THETA ** (np.arange(0, D, 2, dtype=np.float32) / D))
    ang = np.arange(S, dtype=np.float32)[:, None] * freqs[None, :]  # [S, D/2]
    cos = np.cos(ang).astype(np.float32)
    sin = np.sin(ang).astype(np.float32)
    sgn = np.where(np.arange(128) % 2 == 0, -1.0, 1.0).astype(np.float32)

    # packed constants: psw | ident | onesk | mask1
    pswap = np.zeros((128, 128), np.float32)
    idx = np.arange(128)
    pswap[idx, idx ^ 1] = 1.0
    p = np.arange(128)[:, None]
    f = np.arange(512)[None, :]
    mask1 = np.where(p > f, np.float32(NEG), np.float32(0.0))
    constd = np.concatenate(
        [pswap, np.eye(128, dtype=np.float32),
         np.ones((128, 1), np.float32), mask1], axis=1).astype(BF)

    xrs = [np.ascontiguousarray(
        x[b].reshape(N_SC, 512, N_DT, 128).transpose(3, 0, 2, 1)).astype(BF)
        for b in range(B)]

    in_maps = []
    for c in range(8):
        b, g = divmod(c, 4)
        wqr = Wq[512 * g:512 * (g + 1)].reshape(512, N_DT, 128).transpose(
            2, 1, 0).astype(BF)
        wkr = Wk[128 * g:128 * (g + 1)].reshape(128, N_DT, 128).transpose(
            2, 1, 0).astype(BF)
        wvr = Wv[128 * g:128 * (g + 1)].reshape(128, N_DT, 128).transpose(
            2, 1, 0).astype(BF)
        wor = Wo[:, 512 * g:512 * (g + 1)].reshape(S, HPC, 128).transpose(
            2, 1, 0).astype(BF)
        rar = np.empty((128, N_SC, HPC, 512), np.float32)
        rbr = np.empty((128, N_SC, HPC, 512), np.float32)
        for i in range(HPC):
            fidx = 256 * g + 64 * i + (np.arange(128) // 2)  # [128]
            band_a = cos[:, fidx].T  # [128, S]
            band_b = sin[:, fidx].T * sgn[:, None]
            rar[:, :, i, :] = band_a.reshape(128, N_SC, 512)
            rbr[:, :, i, :] = band_b.reshape(128, N_SC, 512)
        in_maps.append({
            "xr": xrs[b],
            "wqr": np.ascontiguousarray(wqr),
            "wkr": np.ascontiguousarray(wkr),
            "wvr": np.ascontiguousarray(wvr),
            "wor": np.ascontiguousarray(wor),
            "rar": rar.astype(BF),
            "rbr": rbr.astype(BF),
            "constd": constd,
            "ons2d": np.ones((128, 2, 16), np.float32).astype(F8),
            "ons1d": np.ones((1, 128), np.float32),
        })
    return in_maps


def kernel(x, Wq, Wk, Wv, Wo, mask, _trace=False):
    in_maps = host_inputs(x, Wq, Wk, Wv, Wo)
    nc = get_nc()
    res = run_bass_kernel_spmd(nc, in_maps, list(range(8)), trace=_trace)
    # outd [128, jb, sc, 512] -> partial [D, S]
    outs = [np.asarray(res.results[c]["outd"], dtype=np.float32)
            .transpose(1, 0, 2, 3).reshape(D, S) for c in range(8)]
    out = np.stack([
        (outs[4 * b + 0] + outs[4 * b + 1] + outs[4 * b + 2]
         + outs[4 * b + 3]).T
        for b in range(B)
    ]).astype(np.float32)
    if _trace:
        kernel.last_result = res
    return out


# revision 14
# speedup vs baseline: 1.3042x; 1.0218x over previous
"""GQA attention kernel for 8 Trainium2 NeuronCores (v3, bf16 + fp8 PV).

Problem: B=2, S=2048, D=2048, 16 q-heads / 4 kv-heads (GQA), head_dim=128,
causal mask, RoPE over the full hidden dim (each head rotates with its own
frequency band), scale 1/sqrt(D), output projection.

Sharding: core c = 4*b + g handles batch b (of 2) and head-group g (of 4):
q-heads 4g..4g+3, which all share kv-head g.  The only cross-core reduction
is the output projection, summed on the host over the 4 head-groups.

v3 changes vs v2 (350us):
  - startup: one packed constants DMA; x chunk-0 split into graded pieces
    interleaved with the wq halves on the sync queue; wo load deferred to
    chunk 1; rope chunk 0 on the gpsimd queue.  First matmul ~9us, not 27.
  - off-diagonal PV + probs-sum matmuls run in fp8e4 with
    perf_mode=DoubleRow (two key-blocks contracted per matmul, 2x rate);
    the exp writes those probs tiles directly in fp8.  Diagonal blocks stay
    bf16 with trimmed widths.  PV and the sum share the same fp8 probs, so
    the normalization stays consistent.
  - causal mask add back on DVE (PE is the bottleneck); the softmax
    reciprocal moved to the scalar engine as exp(-ln(sum)) (both functions
    live in one activation table set), so no multi-us op ever sits in the
    DVE FIFO ahead of the mask adds.
"""

import sys

sys.path.insert(0, "/opt/trn_rl_repo")

from contextlib import ExitStack

import ml_dtypes
import numpy as np

import concourse.bass as bass
import concourse.tile as tile
from concourse.tile import add_dep_helper
from concourse import bacc, mybir
from concourse.bass_utils import run_bass_kernel_spmd

B, S, D = 2, 2048, 2048
NH, NG = 16, 4
KVH = NH // NG  # 4
HD = D // NH  # 128
HPC = 4  # q heads per core
ROPE_THETA = 10000.0
INV_SQRT_D = 1.0 / float(np.sqrt(np.float32(D)))
NEG = -1.0e30

F32 = mybir.dt.float32
F32R = mybir.dt.float32r
BF16 = mybir.dt.bfloat16
FP8 = mybir.dt.float8e4
BF = ml_dtypes.bfloat16
F8 = ml_dtypes.float8_e4m3

N_DT = D // 128  # 16 contraction tiles
N_SC = S // 512  # 4 seq chunks of 512
N_SB = S // 128  # 16 seq blocks of 128

SWAP_MASK = [j ^ 1 for j in range(32)]
EXP = mybir.ActivationFunctionType.Exp
LN = mybir.ActivationFunctionType.Ln
DR = mybir.MatmulPerfMode.DoubleRow


def build_kernel_body(ctx: ExitStack, tc: tile.TileContext, outd, ins):
    nc = tc.nc
    xr, wqr, wkr, wvr, wor, rar, rbr, constd, ons2d, ons1d = ins

    # ---------------- persistent tiles + early DMAs ----------------
    persist = ctx.enter_context(tc.tile_pool(name="persist", bufs=1))
    qt_sb = persist.tile([128, HPC, S], BF16)  # Q^T roped, per head
    kt_sb = persist.tile([128, HPC, S], BF16)  # K^T roped, per band
    v_sb = persist.tile([128, N_SB, 128], BF16)  # V s-major (diag blocks)
    v_f8 = persist.tile([128, N_SB // 2, 2, 128], FP8)  # V pairs (off-diag)
    at_sb = persist.tile([128, HPC, S], BF16)  # attn^T per head
    wo_sb = persist.tile([128, HPC, S], BF16)
    const_sb = persist.tile([128, 769], BF16)  # psw|ident|onesk|mask1
    zbias = persist.tile([128, 1], F32)
    ones2 = persist.tile([128, 2, 16], FP8)
    ones1 = persist.tile([1, 128], F32R)

    psw_sb = const_sb[:, 0:128]
    ident = const_sb[:, 128:256]
    onesk = const_sb[:, 256:257]
    mask1 = const_sb[:, 257:769]

    nc.gpsimd.memset(zbias[:], 0.0)
    nc.gpsimd.dma_start(const_sb[:], constd[:])
    nc.gpsimd.dma_start(ones2[:], ons2d[:])
    nc.gpsimd.dma_start(ones1[:], ons1d[:])

    # ---------------- phase 1: projections + RoPE ----------------
    with tc.tile_pool(name="proj_w", bufs=1) as wpool, \
         tc.tile_pool(name="xc", bufs=2) as xcpool, \
         tc.tile_pool(name="ra", bufs=2) as rapool, \
         tc.tile_pool(name="rb", bufs=2) as rbpool, \
         tc.tile_pool(name="kev", bufs=2) as kevpool, \
         tc.tile_pool(name="vev", bufs=2) as vevpool, \
         tc.tile_pool(name="qev", bufs=3) as qevpool, \
         tc.tile_pool(name="sws", bufs=3) as swspool, \
         tc.tile_pool(name="tmp", bufs=4) as tmppool, \
         tc.tile_pool(name="pacc_kv", bufs=1, space="PSUM") as pkv, \
         tc.tile_pool(name="pacc_q", bufs=1, space="PSUM") as pq, \
         tc.tile_pool(name="pswq", bufs=2, space="PSUM") as pswq_pool:

        wk_sb = wpool.tile([128, N_DT, 128], BF16)
        wv_sb = wpool.tile([128, N_DT, 128], BF16)
        wq_sb = wpool.tile([128, N_DT, 512], BF16)
        nc.sync.dma_start(wk_sb[:], wkr[:])
        nc.scalar.dma_start(wv_sb[:], wvr[:])

        # chunk-0 x tiles in graded pieces, wq halves interleaved
        xcs = []
        xc0 = xcpool.tile([128, N_DT, 512], BF16, name="xc_t")
        nc.sync.dma_start(xc0[:, 0:2, :], xr[:, 0, 0:2, :])
        nc.sync.dma_start(xc0[:, 2:6, :], xr[:, 0, 2:6, :])
        nc.sync.dma_start(wq_sb[:, 0:8, :], wqr[:, 0:8, :])
        nc.sync.dma_start(xc0[:, 6:12, :], xr[:, 0, 6:12, :])
        nc.sync.dma_start(wq_sb[:, 8:16, :], wqr[:, 8:16, :])
        nc.sync.dma_start(xc0[:, 12:16, :], xr[:, 0, 12:16, :])
        xcs.append(xc0)
        ra0 = rapool.tile([128, HPC, 512], BF16, name="ra_t")
        rb0 = rbpool.tile([128, HPC, 512], BF16, name="rb_t")
        nc.gpsimd.dma_start(ra0[:], rar[:, 0])
        nc.gpsimd.dma_start(rb0[:], rbr[:, 0])
        ras, rbs = [ra0], [rb0]

        for c in range(N_SC):
            cs = slice(512 * c, 512 * (c + 1))
            xc = xcs[c]
            if c + 1 < N_SC:
                xcn = xcpool.tile([128, N_DT, 512], BF16, name="xc_t")
                nc.sync.dma_start(xcn[:], xr[:, c + 1])
                xcs.append(xcn)
            ra, rb = ras[c], rbs[c]

            # K/V pass (2 psum banks)
            kv_ps = pkv.tile([128, 2, 512], F32)
            for dt in range(N_DT):
                st, sp = dt == 0, dt == N_DT - 1
                nc.tensor.matmul(kv_ps[:, 0, :], wk_sb[:, dt, :],
                                 xc[:, dt, :], start=st, stop=sp)
                nc.tensor.matmul(kv_ps[:, 1, :], wv_sb[:, dt, :],
                                 xc[:, dt, :], start=st, stop=sp)
            kraw = kevpool.tile([128, 512], BF16)
            kraw_i = nc.scalar.copy(kraw[:], kv_ps[:, 0, :])
            vtr = vevpool.tile([128, 512], BF16)
            nc.scalar.copy(vtr[:], kv_ps[:, 1, :])
            if c + 1 < N_SC:
                ran = rapool.tile([128, HPC, 512], BF16, name="ra_t")
                rbn = rbpool.tile([128, HPC, 512], BF16, name="rb_t")
                d1 = nc.scalar.dma_start(ran[:], rar[:, c + 1])
                d2 = nc.scalar.dma_start(rbn[:], rbr[:, c + 1])
                add_dep_helper(d1.ins, kraw_i.ins, sync=False,
                               reason="defer rope prefetch issue")
                add_dep_helper(d2.ins, kraw_i.ins, sync=False,
                               reason="defer rope prefetch issue")
                ras.append(ran)
                rbs.append(rbn)
            if c == 1:
                dwo = nc.scalar.dma_start(wo_sb[:], wor[:])
                add_dep_helper(dwo.ins, kraw_i.ins, sync=False,
                               reason="defer wo load issue")

            # Q pass (4 psum banks)
            q_ps = pq.tile([128, HPC, 512], F32)
            for dt in range(N_DT):
                st, sp = dt == 0, dt == N_DT - 1
                for i in range(HPC):
                    nc.tensor.matmul(
                        q_ps[:, i, :], wq_sb[:, dt, 128 * i:128 * (i + 1)],
                        xc[:, dt, :], start=st, stop=sp)

            # V: transpose hd-major -> s-major; dual bf16 + fp8 copies
            for j in range(4):
                kb = 4 * c + j
                vt_ps = pswq_pool.tile([128, 128], BF16, name="swq_t")
                nc.tensor.transpose(vt_ps[:],
                                    vtr[:, 128 * j:128 * (j + 1)], ident)
                nc.scalar.copy(v_sb[:, kb, :], vt_ps[:])
                nc.scalar.copy(v_f8[:, kb // 2, kb % 2, :], vt_ps[:])

            # K swap (pair-exchange along partitions) via DVE shuffle
            ksw = swspool.tile([128, 512], BF16, name="sw_t")
            nc.vector.stream_shuffle(ksw[:], kraw[:], SWAP_MASK)

            # RoPE per head/band
            for i in range(HPC):
                qraw = qevpool.tile([128, 512], BF16)
                nc.scalar.copy(qraw[:], q_ps[:, i, :])
                qsw = swspool.tile([128, 512], BF16, name="sw_t")
                nc.vector.stream_shuffle(qsw[:], qraw[:], SWAP_MASK)

                t1 = tmppool.tile([128, 512], BF16)
                nc.vector.tensor_mul(t1[:], qraw[:], ra[:, i, :])
                t2 = tmppool.tile([128, 512], BF16)
                nc.vector.tensor_mul(t2[:], qsw[:], rb[:, i, :])
                nc.gpsimd.tensor_add(qt_sb[:, i, cs], t1[:], t2[:])

                t3 = tmppool.tile([128, 512], BF16)
                nc.vector.tensor_mul(t3[:], kraw[:], ra[:, i, :])
                t4 = tmppool.tile([128, 512], BF16)
                nc.vector.tensor_mul(t4[:], ksw[:], rb[:, i, :])
                nc.gpsimd.tensor_add(kt_sb[:, i, cs], t3[:], t4[:])

    # ---------------- phase 2: attention ----------------
    with tc.tile_pool(name="ptp", bufs=3) as ptppool, \
         tc.tile_pool(name="ptd", bufs=3) as ptdpool, \
         tc.tile_pool(name="lnv", bufs=2) as lnpool, \
         tc.tile_pool(name="rcp", bufs=2) as rcppool, \
         tc.tile_pool(name="bcs", bufs=2) as bcspool, \
         tc.tile_pool(name="ovs", bufs=3) as ovspool, \
         tc.tile_pool(name="st_ps", bufs=3, space="PSUM") as stpool, \
         tc.tile_pool(name="ov_ps", bufs=2, space="PSUM") as ovpool, \
         tc.tile_pool(name="nrm_ps", bufs=3, space="PSUM") as nrmpool:

        # Deferred normalization: stage A (evacuate ov, ln of sums, then
        # exp(-ln) -> 1/sum, all off the PE) one iteration later; stage B
        # (PE broadcast + final at_sb multiply) three iterations later.
        stage_a, stage_b = [], []

        def emit_stage_a():
            if stage_a:
                stage_a.pop(0)()

        def emit_stage_b(min_pending):
            while len(stage_b) > min_pending:
                stage_b.pop(0)()

        for qc in range(N_SC):
            for i in range(HPC):
                nkb = 4 * (qc + 1)
                npair = (4 * qc) // 2  # off-diagonal key-block pairs
                nunit = npair + 4
                ov_ps = ovpool.tile([128, 512], F32)
                sum_ps = nrmpool.tile([16, 512], F32, name="nrm_t")

                emit_stage_a()

                def emit_pair(m, pt2, ov_ps=ov_ps, sum_ps=sum_ps):
                    st = m == 0
                    nc.tensor.matmul(ov_ps[:], v_f8[:, m, :, :],
                                     pt2[:], perf_mode=DR,
                                     start=st, stop=False)
                    nc.tensor.matmul(sum_ps[:], ones2[:],
                                     pt2[:], perf_mode=DR,
                                     start=st, stop=False)

                def emit_diag(kb, ptd, qc=qc, ov_ps=ov_ps, sum_ps=sum_ps,
                              nkb=nkb):
                    o = kb - 4 * qc
                    qo = 128 * o
                    n = 512 - qo
                    st = kb == 0
                    sp = kb == nkb - 1
                    nc.tensor.matmul(ov_ps[:, qo:], v_sb[:, kb, :],
                                     ptd[:, :n], start=st, stop=sp)
                    nc.tensor.matmul(sum_ps[0:1, qo:], onesk,
                                     ptd[:, :n], start=st, stop=sp)

                prev = None  # closure awaiting PV/SUM emission
                for u in range(nunit):
                    if u < npair:
                        pt2 = ptppool.tile([128, 2, 512], FP8)
                        for j in range(2):
                            kb = 2 * u + j
                            st_ps = stpool.tile([128, 512], F32)
                            nc.tensor.matmul(
                                st_ps[:],
                                kt_sb[:, i, 128 * kb:128 * (kb + 1)],
                                qt_sb[:, i, 512 * qc:512 * (qc + 1)],
                                start=True, stop=True)
                            nc.scalar.activation(
                                pt2[:, j, :], st_ps[:], EXP,
                                bias=zbias[:], scale=INV_SQRT_D)
                        cur = (lambda u=u, pt2=pt2:
                               emit_pair(u, pt2))
                    else:
                        kb = 4 * qc + (u - npair)
                        o = u - npair
                        qo = 128 * o
                        n = 512 - qo
                        st_ps = stpool.tile([128, 512], F32)
                        nc.tensor.matmul(
                            st_ps[:, :n],
                            kt_sb[:, i, 128 * kb:128 * (kb + 1)],
                            qt_sb[:, i, 512 * qc + qo:512 * (qc + 1)],
                            start=True, stop=False)
                        nc.tensor.matmul(st_ps[:, :n], ident,
                                         mask1[:, :n], start=False, stop=True)
                        ptd = ptdpool.tile([128, 512], BF16)
                        nc.scalar.activation(
                            ptd[:, :n], st_ps[:, :n], EXP,
                            bias=zbias[:], scale=INV_SQRT_D)
                        cur = (lambda kb=kb, ptd=ptd:
                               emit_diag(kb, ptd))
                    if u == 1:
                        emit_stage_b(2)
                    if prev is not None:
                        prev()
                    prev = cur
                prev()

                def a_step(i=i, qc=qc, ov_ps=ov_ps, sum_ps=sum_ps):
                    ovS = ovspool.tile([128, 512], BF16)
                    nc.vector.tensor_copy(ovS[:], ov_ps[:])
                    sumS = lnpool.tile([1, 512], F32)
                    nc.vector.tensor_copy(sumS[:], sum_ps[0:1, :])
                    rcp = rcppool.tile([1, 512], F32R)
                    with nc.allow_low_precision(
                            reason="f32r view of fp32 for matmul rhs"):
                        nc.vector.reciprocal(rcp[:], sumS[:])

                    def b_step(i=i, qc=qc, ovS=ovS, rcp=rcp):
                        bc_ps = nrmpool.tile([128, 512], F32, name="nrm_t")
                        nc.tensor.matmul(bc_ps[:], ones1[:], rcp[:],
                                         start=True, stop=True)
                        bcS = bcspool.tile([128, 512], BF16)
                        nc.vector.tensor_copy(bcS[:], bc_ps[:])
                        nc.vector.tensor_mul(
                            at_sb[:, i, 512 * qc:512 * (qc + 1)],
                            ovS[:], bcS[:])

                    stage_b.append(b_step)

                stage_a.append(a_step)

        emit_stage_a()
        emit_stage_b(0)

    # ---------------- phase 3: output projection ----------------
    with tc.tile_pool(name="osb", bufs=3) as opool, \
         tc.tile_pool(name="op_ps", bufs=3, space="PSUM") as oppool:
        for sc in range(N_SC):
            ss = slice(512 * sc, 512 * (sc + 1))
            for jb in range(N_SB):
                op_ps = oppool.tile([128, 512], F32)
                for h in range(HPC):
                    nc.tensor.matmul(
                        op_ps[:], wo_sb[:, h, 128 * jb:128 * (jb + 1)],
                        at_sb[:, h, ss],
                        start=(h == 0), stop=(h == HPC - 1))
                osb = opool.tile([128, 512], BF16)
                if jb % 2 == 0:
                    nc.scalar.copy(osb[:], op_ps[:])
                else:
                    nc.vector.tensor_copy(osb[:], op_ps[:])
                nc.sync.dma_start(outd[:, jb, sc, :], osb[:])


_NC_CACHE = None


def get_nc():
    global _NC_CACHE
    if _NC_CACHE is not None:
        return _NC_CACHE
    nc = bacc.Bacc("TRN2", target_bir_lowering=False, debug=False,
                   num_devices=8)
    xr = nc.dram_tensor("xr", [128, N_SC, N_DT, 512], BF16,
                        kind="ExternalInput").ap()
    wqr = nc.dram_tensor("wqr", [128, N_DT, 512], BF16,
                         kind="ExternalInput").ap()
    wkr = nc.dram_tensor("wkr", [128, N_DT, 128], BF16,
                         kind="ExternalInput").ap()
    wvr = nc.dram_tensor("wvr", [128, N_DT, 128], BF16,
                         kind="ExternalInput").ap()
    wor = nc.dram_tensor("wor", [128, HPC, S], BF16,
                         kind="ExternalInput").ap()
    rar = nc.dram_tensor("rar", [128, N_SC, HPC, 512], BF16,
                         kind="ExternalInput").ap()
    rbr = nc.dram_tensor("rbr", [128, N_SC, HPC, 512], BF16,
                         kind="ExternalInput").ap()
    constd = nc.dram_tensor("constd", [128, 769], BF16,
                            kind="ExternalInput").ap()
    ons2d = nc.dram_tensor("ons2d", [128, 2, 16], FP8,
                           kind="ExternalInput").ap()
    ons1d = nc.dram_tensor("ons1d", [1, 128], F32R,
                           kind="ExternalInput").ap()
    outd = nc.dram_tensor("outd", [128, N_SB, N_SC, 512], BF16,
                          kind="ExternalOutput").ap()

    with tile.TileContext(nc) as tc, ExitStack() as ctx:
        build_kernel_body(ctx, tc, outd,
                          (xr, wqr, wkr, wvr, wor, rar, rbr, constd,
                           ons2d, ons1d))
    nc.compile()
    _NC_CACHE = nc
    return nc


def host_inputs(x, Wq, Wk, Wv, Wo):
    """Per-core input dicts (core c = 4*b + g), pre-arranged + cast."""
    x = np.asarray(x, np.float32)
    Wq = np.asarray(Wq, np.float32)
    Wk = np.asarray(Wk, np.float32)
    Wv = np.asarray(Wv, np.float32)
    Wo = np.asarray(Wo, np.float32)

    # rope tables (same freqs layout as the reference)
    freqs = 1.0 / (ROPE_THETA ** (np.arange(0, D, 2, dtype=np.float32) / D))
    ang = np.arange(S, dtype=np.float32)[:, None] * freqs[None, :]  # [S, D/2]
    cos = np.cos(ang).astype(np.float32)
    sin = np.sin(ang).astype(np.float32)
    sgn = np.where(np.arange(128) % 2 == 0, -1.0, 1.0).astype(np.float32)

    # packed constants: psw | ident | onesk | mask1
    pswap = np.zeros((128, 128), np.float32)
    idx = np.arange(128)
    pswap[idx, idx ^ 1] = 1.0
    p = np.arange(128)[:, None]
    f = np.arange(512)[None, :]
    mask1 = np.where(p > f, np.float32(NEG), np.float32(0.0))
    constd = np.concatenate(
        [pswap, np.eye(128, dtype=np.float32),
         np.ones((128, 1), np.float32), mask1], axis=1).astype(BF)

    xrs = [np.ascontiguousarray(
        x[b].reshape(N_SC, 512, N_DT, 128).transpose(3, 0, 2, 1)).astype(BF)
        for b in range(B)]

    in_maps = []
    for c in range(8):
        b, g = divmod(c, 4)
        wqr = Wq[512 * g:512 * (g + 1)].reshape(512, N_DT, 128).transpose(
            2, 1, 0).astype(BF)
        wkr = Wk[128 * g:128 * (g + 1)].reshape(128, N_DT, 128).transpose(
            2, 1, 0).astype(BF)
        wvr = Wv[128 * g:128 * (g + 1)].reshape(128, N_DT, 128).transpose(
            2, 1, 0).astype(BF)
        wor = Wo[:, 512 * g:512 * (g + 1)].reshape(S, HPC, 128).transpose(
            2, 1, 0).astype(BF)
        rar = np.empty((128, N_SC, HPC, 512), np.float32)
        rbr = np.empty((128, N_SC, HPC, 512), np.float32)
        for i in range(HPC):
            fidx = 256 * g + 64 * i + (np.arange(128) // 2)  # [128]
            band_a = cos[:, fidx].T  # [128, S]
            band_b = sin[:, fidx].T * sgn[:, None]
            rar[:, :, i, :] = band_a.reshape(128, N_SC, 512)
            rbr[:, :, i, :] = band_b.reshape(128, N_SC, 512)
        in_maps.append({
            "xr": xrs[b],
            "wqr": np.ascontiguousarray(wqr),
            "wkr": np.ascontiguousarray(wkr),
            "wvr": np.ascontiguousarray(wvr),
            "wor": np.ascontiguousarray(wor),
            "rar": rar.astype(BF),
            "rbr": rbr.astype(BF),
            "constd": constd,
            "ons2d": np.ones((128, 2, 16), np.float32).astype(F8),
            "ons1d": np.ones((1, 128), np.float32),
        })
    return in_maps


def kernel(x, Wq, Wk, Wv, Wo, mask, _trace=False):
    in_maps = host_inputs(x, Wq, Wk, Wv, Wo)
    nc = get_nc()
    res = run_bass_kernel_spmd(nc, in_maps, list(range(8)), trace=_trace)
    # outd [128, jb, sc, 512] -> partial [D, S]
    outs = [np.asarray(res.results[c]["outd"], dtype=np.float32)
            .transpose(1, 0, 2, 3).reshape(D, S) for c in range(8)]
    out = np.stack([
        (outs[4 * b + 0] + outs[4 * b + 1] + outs[4 * b + 2]
         + outs[4 * b + 3]).T
        for b in range(B)
    ]).astype(np.float32)
    if _trace:
        kernel.last_result = res
    return out
